# revision 1
# baseline (speedup 1.0000x reference)
"""Trainium2 Bass kernel for masked 3D-GIoU regression loss (262144 box pairs).

Per core (8 cores, data-parallel over boxes): 32768 boxes as 128 partitions x
256 free elements. All geometry is elementwise plane ops:

  - BEV rotated-rect intersection via Liang-Barsky clipping + Green's theorem
    (no argsort; identical to the reference's angle-sort shoelace for
    non-degenerate inputs -- validated to 1e-14 per box in f64).
  - Smallest enclosing rectangle: min over 20 candidate directions
    (4 rect edge dirs + 16 corner cross-pairs). Equals the reference's
    28-pair min by the rotating-calipers theorem (diagonals are never hull
    edges of the union).
  - Device reduces to per-partition partial sums; host sums 8x128 partials
    and divides once.
"""

import sys
import numpy as np

if "/opt/trn_rl_repo" not in sys.path:
    sys.path.insert(0, "/opt/trn_rl_repo")

import concourse.bacc as bacc  # noqa: E402
import concourse.mybir as mybir  # noqa: E402
import concourse.tile as tile  # noqa: E402
from concourse import bass_utils  # noqa: E402
from concourse.alu_op_type import AluOpType as OP  # noqa: E402

N_CORES = 8
N_TOTAL = 262144
N_CORE = N_TOTAL // N_CORES  # 32768
P = 128
F = N_CORE // P  # 256
FP = mybir.dt.float32
ACTF = mybir.ActivationFunctionType
PI = float(np.pi)

# rotating temp-tag classes: tag -> (free elems, bufs)
_CLS = {
    "tF": (F, 18),
    "t4F": (4 * F, 11),
    "t8F": (8 * F, 5),
}


def _build():
    nc = bacc.Bacc("TRN2", target_bir_lowering=False, debug=False)
    pred_d = nc.dram_tensor("pred", [N_CORE, 7], FP, kind="ExternalInput")
    tgt_d = nc.dram_tensor("target", [N_CORE, 7], FP, kind="ExternalInput")
    iou_d = nc.dram_tensor("iou", [N_CORE], FP, kind="ExternalInput")
    out_d = nc.dram_tensor("partials", [P, 2], FP, kind="ExternalOutput")

    V = nc.vector
    G = nc.gpsimd
    S = nc.scalar

    uid = [0]

    def mk(pool, cls):
        def t(_tag=None):
            uid[0] += 1
            fe, bufs = _CLS[cls]
            return pool.tile([P, fe], FP, tag=cls, bufs=bufs,
                             name=f"{cls}_{uid[0]}")[:]
        return t

    with tile.TileContext(nc) as tc:
        with tc.tile_pool(name="pers", bufs=1) as pers:
            def PT(tag, shape=None):
                return pers.tile(shape or [P, F], FP, tag=tag, name=tag)[:]

            def view(ap, g):
                return ap.rearrange("p (g f) -> p g f", g=g)

            def bc(plane, g):
                return plane.rearrange("p (o f) -> p o f", o=1).broadcast_to([P, g, F])

            halfpi = PT("halfpi", [P, 1])
            V.memset(halfpi, PI / 2)

            consts = {}

            def cplane(c):
                if c not in consts:
                    t = pers.tile([P, 1], FP, tag=f"c_{len(consts)}",
                                  name=f"c_{len(consts)}")[:]
                    V.memset(t, float(c))
                    consts[c] = t
                return consts[c]

            def cb(c, like):
                t = cplane(c)
                shp = list(like.shape)
                pat = "p (" + " ".join(f"d{i}" for i in range(len(shp) - 1)) + ") -> p " +                       " ".join(f"d{i}" for i in range(len(shp) - 1))
                kw = {f"d{i}": 1 for i in range(len(shp) - 1)}
                return t.rearrange(pat, **kw).broadcast_to(shp)

            def g_ts(out, in_, c, op):
                G.tensor_tensor(out, in_, cb(c, out), op=op)

            # ======== stage 1 (scoped pool; frees before main work) ========
            with tc.tile_pool(name="stage1", bufs=1) as p1:
                tF1 = mk(p1, "tF")

                predI = p1.tile([P, 7 * F], FP, tag="predI", name="predI")[:]
                tgtI = p1.tile([P, 7 * F], FP, tag="tgtI", name="tgtI")[:]
                iouP = tF1()
                nc.sync.dma_start(predI, pred_d.ap().rearrange("(p f) c -> p (f c)", p=P))
                nc.sync.dma_start(tgtI, tgt_d.ap().rearrange("(p f) c -> p (f c)", p=P))
                nc.sync.dma_start(iouP, iou_d.ap().rearrange("(p f) -> p f", p=P))

                pv = predI.rearrange("p (f c) -> p c f", c=7)
                tv = tgtI.rearrange("p (f c) -> p c f", c=7)
                x1, y1, z1, w1, l1, h1, yaw1 = (pv[:, c, :] for c in range(7))
                x2, y2, z2, w2, l2, h2, yaw2 = (tv[:, c, :] for c in range(7))

                def sincos(eng, yaw, pfx):
                    is_v = eng is V
                    g1 = tF1()
                    V.tensor_scalar(g1, yaw, PI, None, op0=OP.is_gt)
                    g2 = tF1()
                    V.tensor_scalar(g2, yaw, -PI, None, op0=OP.is_lt)
                    adj = tF1()
                    eng.tensor_tensor(adj, g2, g1, op=OP.subtract)
                    yr = tF1()
                    if is_v:
                        eng.scalar_tensor_tensor(yr, adj, 2 * PI, yaw, op0=OP.mult, op1=OP.add)
                    else:
                        tmp = tF1()
                        eng.tensor_tensor(tmp, adj, cb(2 * PI, tmp), op=OP.mult)
                        eng.tensor_tensor(yr, tmp, yaw, op=OP.add)
                    sa = PT(pfx + "sa")
                    S.activation(sa, yr, ACTF.Sin)
                    g3 = tF1()
                    V.tensor_scalar(g3, yr, PI / 2, None, op0=OP.is_gt)
                    yc = tF1()
                    if is_v:
                        eng.scalar_tensor_tensor(yc, g3, -2 * PI, yr, op0=OP.mult, op1=OP.add)
                    else:
                        tmp2 = tF1()
                        eng.tensor_tensor(tmp2, g3, cb(-2 * PI, tmp2), op=OP.mult)
                        eng.tensor_tensor(yc, tmp2, yr, op=OP.add)
                    ca = PT(pfx + "ca")
                    S.activation(ca, yc, ACTF.Sin, bias=halfpi)
                    return sa, ca

                sa1, ca1 = sincos(V, yaw1, "t1")
                sa2, ca2 = sincos(G, yaw2, "t2")

                cx2 = PT("cx2")
                G.tensor_tensor(cx2, x2, x1, op=OP.subtract)
                cy2 = PT("cy2")
                G.tensor_tensor(cy2, y2, y1, op=OP.subtract)

                def halfaxes(eng, w, l, sa, ca, r):
                    if eng is V:
                        A = PT(f"A{r}")
                        eng.scalar_tensor_tensor(A, w, 0.5, ca, op0=OP.mult, op1=OP.mult)
                        B = PT(f"B{r}")
                        eng.scalar_tensor_tensor(B, w, 0.5, sa, op0=OP.mult, op1=OP.mult)
                        C = tF1()
                        eng.scalar_tensor_tensor(C, l, 0.5, sa, op0=OP.mult, op1=OP.mult)
                        D = PT(f"D{r}")
                        eng.scalar_tensor_tensor(D, l, 0.5, ca, op0=OP.mult, op1=OP.mult)
                    else:
                        hw = tF1()
                        eng.tensor_tensor(hw, w, cb(0.5, hw), op=OP.mult)
                        hl = tF1()
                        eng.tensor_tensor(hl, l, cb(0.5, hl), op=OP.mult)
                        A = PT(f"A{r}")
                        eng.tensor_tensor(A, hw, ca, op=OP.mult)
                        B = PT(f"B{r}")
                        eng.tensor_tensor(B, hw, sa, op=OP.mult)
                        C = tF1()
                        eng.tensor_tensor(C, hl, sa, op=OP.mult)
                        D = PT(f"D{r}")
                        eng.tensor_tensor(D, hl, ca, op=OP.mult)
                    nC = PT(f"nC{r}")
                    S.mul(nC, C, -1.0)
                    Pp = PT(f"P{r}")
                    eng.tensor_tensor(Pp, A, C, op=OP.subtract)
                    Q = PT(f"Q{r}")
                    eng.tensor_tensor(Q, B, D, op=OP.add)
                    R = PT(f"R{r}")
                    eng.tensor_tensor(R, A, C, op=OP.add)
                    Ss = PT(f"S{r}")
                    eng.tensor_tensor(Ss, B, D, op=OP.subtract)
                    return A, B, D, nC, Pp, Q, R, Ss

                A1, B1, D1, nC1, P1, Q1, R1, S1 = halfaxes(V, w1, l1, sa1, ca1, 1)
                A2, B2, D2, nC2, P2, Q2, R2, S2 = halfaxes(G, w2, l2, sa2, ca2, 2)

                # z overlap / volumes / mask
                hh1 = tF1()
                g_ts(hh1, h1, 0.5, OP.mult)
                hh2 = tF1()
                g_ts(hh2, h2, 0.5, OP.mult)
                zmax1 = tF1()
                G.tensor_tensor(zmax1, z1, hh1, op=OP.add)
                zmin1 = tF1()
                G.tensor_tensor(zmin1, z1, hh1, op=OP.subtract)
                zmax2 = tF1()
                G.tensor_tensor(zmax2, z2, hh2, op=OP.add)
                zmin2 = tF1()
                G.tensor_tensor(zmin2, z2, hh2, op=OP.subtract)
                mn_hi = tF1()
                V.tensor_tensor(mn_hi, zmax1, zmax2, op=OP.min)
                mx_lo = tF1()
                V.tensor_tensor(mx_lo, zmin1, zmin2, op=OP.max)
                ozr = tF1()
                G.tensor_tensor(ozr, mn_hi, mx_lo, op=OP.subtract)
                oz = PT("oz")
                V.tensor_scalar(oz, ozr, 0.0, None, op0=OP.max)
                mx_hi = tF1()
                V.tensor_tensor(mx_hi, zmax1, zmax2, op=OP.max)
                mn_lo = tF1()
                V.tensor_tensor(mn_lo, zmin1, zmin2, op=OP.min)
                zrr = tF1()
                G.tensor_tensor(zrr, mx_hi, mn_lo, op=OP.subtract)
                zr = PT("zr")
                V.tensor_scalar(zr, zrr, 0.0, None, op0=OP.max)

                v1a = tF1()
                G.tensor_tensor(v1a, w1, l1, op=OP.mult)
                v1v = PT("v1v")
                G.tensor_tensor(v1v, v1a, h1, op=OP.mult)
                v2a = tF1()
                G.tensor_tensor(v2a, w2, l2, op=OP.mult)
                v2v = PT("v2v")
                G.tensor_tensor(v2v, v2a, h2, op=OP.mult)
                mask = PT("mask")
                V.tensor_scalar(mask, iouP, 0.55, None, op0=OP.is_ge)

                hw1sq = PT("hw1sq")
                V.scalar_tensor_tensor(hw1sq, w1, 0.25, w1, op0=OP.mult, op1=OP.mult)
                hl1sq = PT("hl1sq")
                V.scalar_tensor_tensor(hl1sq, l1, 0.25, l1, op0=OP.mult, op1=OP.mult)
                hw2sq = PT("hw2sq")
                V.scalar_tensor_tensor(hw2sq, w2, 0.25, w2, op0=OP.mult, op1=OP.mult)
                hl2sq = PT("hl2sq")
                V.scalar_tensor_tensor(hl2sq, l2, 0.25, l2, op0=OP.mult, op1=OP.mult)
                hwl1 = PT("hwl1")
                V.scalar_tensor_tensor(hwl1, w1, 0.25, l1, op0=OP.mult, op1=OP.mult)
                hwl2 = PT("hwl2")
                V.scalar_tensor_tensor(hwl2, w2, 0.25, l2, op0=OP.mult, op1=OP.mult)

                il1 = tF1()
                V.reciprocal(il1, l1)
                rat1 = PT("rat1")
                V.tensor_tensor(rat1, w1, il1, op=OP.mult)
                iw1 = tF1()
                V.reciprocal(iw1, w1)
                irat1 = PT("irat1")
                V.tensor_tensor(irat1, l1, iw1, op=OP.mult)
                il2 = tF1()
                V.reciprocal(il2, l2)
                rat2 = PT("rat2")
                V.tensor_tensor(rat2, w2, il2, op=OP.mult)
                iw2 = tF1()
                V.reciprocal(iw2, w2)
                irat2 = PT("irat2")
                V.tensor_tensor(irat2, l2, iw2, op=OP.mult)

                # ---- dots needed downstream (outputs persistent) ----
                def dot(eng, tag, ax, ay, bx, by):
                    t0 = tF1()
                    eng.tensor_tensor(t0, ax, bx, op=OP.mult)
                    t1 = tF1()
                    eng.tensor_tensor(t1, ay, by, op=OP.mult)
                    o = PT(tag)
                    eng.tensor_tensor(o, t0, t1, op=OP.add)
                    return o

                m_uu = dot(V, "m_uu", A2, B2, A1, B1)
                m_uv = dot(V, "m_uv", A2, B2, nC1, D1)
                m_vu = dot(V, "m_vu", nC2, D2, A1, B1)
                m_vv = dot(V, "m_vv", nC2, D2, nC1, D1)

                def saferec(tag, m):
                    g = tF1()
                    V.tensor_scalar(g, m, 0.0, None, op0=OP.is_ge)
                    s2 = tF1()
                    V.tensor_scalar(s2, g, 2.0, 1.0, op0=OP.mult, op1=OP.subtract)
                    am = PT(tag + "_am")
                    V.scalar_tensor_tensor(am, m, -1.0, m, op0=OP.mult, op1=OP.max)
                    amc = tF1()
                    V.tensor_scalar(amc, am, 1e-12, None, op0=OP.max)
                    ms = tF1()
                    V.tensor_tensor(ms, s2, amc, op=OP.mult)
                    o = PT(tag)
                    V.reciprocal(o, ms)
                    return o, am

                inv_uu, am_uu = saferec("inv_uu", m_uu)
                inv_uv, am_uv = saferec("inv_uv", m_uv)
                inv_vu, am_vu = saferec("inv_vu", m_vu)
                inv_vv, am_vv = saferec("inv_vv", m_vv)

                pj = {}
                for axname, axx, axy, eng in (
                    ("u1", A1, B1, V),
                    ("v1", nC1, D1, V),
                    ("u2", A2, B2, G),
                    ("v2", nC2, D2, G),
                ):
                    for vec, vx, vy in (
                        ("PQ1", P1, Q1),
                        ("RS1", R1, S1),
                        ("PQ2", P2, Q2),
                        ("RS2", R2, S2),
                        ("C", cx2, cy2),
                    ):
                        pj[(axname, vec)] = dot(eng, f"pj_{axname}_{vec}", axx, axy, vx, vy)

                # X_u = cx2*B2 - cy2*A2 ; X_v = cx2*D2 + cy2*C2
                xu0 = tF1()
                G.tensor_tensor(xu0, cx2, B2, op=OP.mult)
                xu1 = tF1()
                G.tensor_tensor(xu1, cy2, A2, op=OP.mult)
                X_u = PT("X_u")
                G.tensor_tensor(X_u, xu0, xu1, op=OP.subtract)
                xv0 = tF1()
                G.tensor_tensor(xv0, cx2, D2, op=OP.mult)
                xv1 = tF1()
                G.tensor_tensor(xv1, cy2, nC2, op=OP.mult)
                X_v = PT("X_vf")
                G.tensor_tensor(X_v, xv0, xv1, op=OP.subtract)  # cx2*D2 + cy2*C2

            # ======== stage 2+: work pool ========
            with tc.tile_pool(name="work", bufs=1) as wp:
                tF = mk(wp, "tF")
                t4F = mk(wp, "t4F")
                t8F = mk(wp, "t8F")

                def absv(eng, a, out=None, mkt=t4F):
                    o = out if out is not None else mkt()
                    eng.scalar_tensor_tensor(o, a, -1.0, a, op0=OP.mult, op1=OP.max)
                    return o

                # ---------------- intersection ----------------
                def corner_su(eng, dPQ, dRS, dC, sign_off):
                    outs = []
                    for (src, sgn) in ((dPQ, 1), (dRS, -1), (dPQ, -1), (dRS, 1)):
                        o = tF()
                        if sign_off < 0:
                            if sgn > 0:
                                eng.tensor_tensor(o, src, dC, op=OP.subtract)
                            else:
                                eng.scalar_tensor_tensor(o, src, -1.0, dC, op0=OP.mult, op1=OP.subtract)
                        else:
                            if sgn > 0:
                                eng.tensor_tensor(o, src, dC, op=OP.add)
                            else:
                                eng.tensor_tensor(o, dC, src, op=OP.subtract)
                        outs.append(o)
                    return outs

                su1u = corner_su(V, pj[("u2", "PQ1")], pj[("u2", "RS1")], pj[("u2", "C")], -1)
                su1v = corner_su(V, pj[("v2", "PQ1")], pj[("v2", "RS1")], pj[("v2", "C")], -1)
                su2u = corner_su(G, pj[("u1", "PQ2")], pj[("u1", "RS2")], pj[("u1", "C")], +1)
                su2v = corner_su(G, pj[("v1", "PQ2")], pj[("v1", "RS2")], pj[("v1", "C")], +1)

                def emit_pass(eng, su_by_axis, h_by_axis, inv_by_edge_axis):
                    is_v = eng is V
                    suA, suB = su_by_axis
                    hA, hB = h_by_axis
                    su_s = t8F()
                    suv = su_s.rearrange("p (e a f) -> p e a f", e=4, a=2)
                    inv_s = t8F()
                    invv = inv_s.rearrange("p (e a f) -> p e a f", e=4, a=2)
                    h_s = t4F()
                    hv = view(h_s, 4)
                    S.copy(hv[:, 0, :], hA)
                    S.copy(hv[:, 1, :], hB)
                    for e in range(4):
                        S.copy(suv[:, e, 0, :], suA[e])
                        S.copy(suv[:, e, 1, :], suB[e])
                        for a in range(2):
                            ip, cf = inv_by_edge_axis[e][a]
                            # for POOL: store NEGATED inv so r1 = (su+h)*(-inv)
                            S.mul(invv[:, e, a, :], ip, cf if is_v else -cf)
                    hb = (h_s[:, 0:2 * F]
                          .rearrange("p (o a f) -> p o a f", o=1, a=2)
                          .broadcast_to([P, 4, 2, F]))
                    sus = su_s.rearrange("p (e a f) -> p e a f", e=4, a=2)
                    a1 = t8F()
                    if is_v:
                        eng.scalar_tensor_tensor(a1.rearrange("p (e a f) -> p e a f", e=4, a=2),
                                                 sus, -1.0, hb, op0=OP.mult, op1=OP.subtract)
                    else:
                        # a1 = su + h ; combined with negated inv gives same r1
                        eng.tensor_tensor(a1.rearrange("p (e a f) -> p e a f", e=4, a=2),
                                          sus, hb, op=OP.add)
                    a2 = t8F()
                    if is_v:
                        eng.tensor_tensor(a2.rearrange("p (e a f) -> p e a f", e=4, a=2),
                                          hb, sus, op=OP.subtract)
                    else:
                        # r2 = (h-su)*inv = (su-h)*(-inv); inv strip holds -inv
                        eng.tensor_tensor(a2.rearrange("p (e a f) -> p e a f", e=4, a=2),
                                          sus, hb, op=OP.subtract)
                    r1 = t8F()
                    eng.tensor_tensor(r1, a1, inv_s, op=OP.mult)
                    r2 = t8F()
                    eng.tensor_tensor(r2, a2, inv_s, op=OP.mult)
                    lo = t8F()
                    eng.tensor_tensor(lo, r1, r2, op=OP.min)
                    hi = t8F()
                    eng.tensor_tensor(hi, r1, r2, op=OP.max)
                    lov = lo.rearrange("p (e a f) -> p e a f", e=4, a=2)
                    hiv = hi.rearrange("p (e a f) -> p e a f", e=4, a=2)
                    t0p = t4F()
                    eng.tensor_tensor(view(t0p, 4), lov[:, :, 0, :], lov[:, :, 1, :], op=OP.max)
                    t0 = t4F()
                    if is_v:
                        eng.tensor_scalar(t0, t0p, 0.0, None, op0=OP.max)
                    else:
                        eng.tensor_tensor(t0, t0p, cb(0.0, t0), op=OP.max)
                    t1p = t4F()
                    eng.tensor_tensor(view(t1p, 4), hiv[:, :, 0, :], hiv[:, :, 1, :], op=OP.min)
                    t1 = t4F()
                    if is_v:
                        eng.tensor_scalar(t1, t1p, 1.0, None, op0=OP.min)
                    else:
                        eng.tensor_tensor(t1, t1p, cb(1.0, t1), op=OP.min)
                    dt = t4F()
                    eng.tensor_tensor(dt, t1, t0, op=OP.subtract)
                    dtc = t4F()
                    if is_v:
                        eng.tensor_scalar(dtc, dt, 0.0, None, op0=OP.max)
                    else:
                        eng.tensor_tensor(dtc, dt, cb(0.0, dtc), op=OP.max)
                    return dtc

                inv1 = [
                    [(inv_uu, -0.5), (inv_vu, -0.5)],
                    [(inv_uv, -0.5), (inv_vv, -0.5)],
                    [(inv_uu, 0.5), (inv_vu, 0.5)],
                    [(inv_uv, 0.5), (inv_vv, 0.5)],
                ]
                dt1 = emit_pass(V, (su1u, su1v), (hw2sq, hl2sq), inv1)
                inv2 = [
                    [(inv_uu, -0.5), (inv_uv, -0.5)],
                    [(inv_vu, -0.5), (inv_vv, -0.5)],
                    [(inv_uu, 0.5), (inv_uv, 0.5)],
                    [(inv_vu, 0.5), (inv_vv, 0.5)],
                ]
                dt2 = emit_pass(V, (su2u, su2v), (hw1sq, hl1sq), inv2)

                dt1v = view(dt1, 4)
                sa_ = t4F()
                V.tensor_tensor(view(sa_, 4)[:, 0:2, :], dt1v[:, 0:2, :], dt1v[:, 2:4, :], op=OP.add)
                sav = view(sa_, 4)
                sum1 = tF()
                V.tensor_tensor(sum1, sav[:, 0, :], sav[:, 1, :], op=OP.add)
                contrib1 = tF()
                V.tensor_tensor(contrib1, sum1, hwl1, op=OP.mult)

                dt2v = view(dt2, 4)
                sb_ = t4F()
                G.tensor_tensor(view(sb_, 4)[:, 0:2, :], dt2v[:, 0:2, :], dt2v[:, 2:4, :], op=OP.add)
                sbv = view(sb_, 4)
                sum2 = tF()
                G.tensor_tensor(sum2, sbv[:, 0, :], sbv[:, 1, :], op=OP.add)
                base2 = tF()
                G.tensor_tensor(base2, sum2, hwl2, op=OP.mult)
                d20 = tF()
                G.tensor_tensor(d20, dt2v[:, 2, :], dt2v[:, 0, :], op=OP.subtract)
                d31 = tF()
                G.tensor_tensor(d31, dt2v[:, 3, :], dt2v[:, 1, :], op=OP.subtract)
                tXu = tF()
                G.tensor_tensor(tXu, d20, X_u, op=OP.mult)
                tXv = tF()
                G.tensor_tensor(tXv, d31, X_v, op=OP.mult)
                c2s = tF()
                G.tensor_tensor(c2s, base2, tXu, op=OP.add)
                c2t = tF()
                G.tensor_tensor(c2t, c2s, tXv, op=OP.add)
                isum = tF()
                V.tensor_tensor(isum, contrib1, c2t, op=OP.add)
                inter2d = PT("inter2d")
                V.scalar_tensor_tensor(inter2d, isum, -1.0, isum, op0=OP.mult, op1=OP.max)

                # ---------------- enclosing ----------------
                ox = wp.tile([P, 4 * F], FP, tag="ox", name="ox")[:]
                oxv = view(ox, 4)
                S.copy(oxv[:, 0, :], P1)
                S.mul(oxv[:, 1, :], R1, -1.0)
                S.mul(oxv[:, 2, :], P1, -1.0)
                S.copy(oxv[:, 3, :], R1)
                oy = wp.tile([P, 4 * F], FP, tag="oy", name="oy")[:]
                oyv = view(oy, 4)
                S.copy(oyv[:, 0, :], Q1)
                S.mul(oyv[:, 1, :], S1, -1.0)
                S.mul(oyv[:, 2, :], Q1, -1.0)
                S.copy(oyv[:, 3, :], S1)
                pos = {}
                for ax in ("u1", "v1", "u2", "v2"):
                    st = wp.tile([P, 4 * F], FP, tag=f"po_{ax}", name=f"po_{ax}")[:]
                    sv = view(st, 4)
                    dPQ1 = pj[(ax, "PQ1")]
                    dRS1 = pj[(ax, "RS1")]
                    S.copy(sv[:, 0, :], dPQ1)
                    S.mul(sv[:, 1, :], dRS1, -1.0)
                    S.mul(sv[:, 2, :], dPQ1, -1.0)
                    S.copy(sv[:, 3, :], dRS1)
                    pos[ax] = sv

                encmin = wp.tile([P, 4 * F], FP, tag="encmin", name="encmin")[:]
                encminv = view(encmin, 4)

                # per-corner-j group of 4 cross directions
                for j in range(4):
                    sP, sR = ((1, 0), (-1, 1), (-1, 0), (1, 1))[j]
                    # rect2 corner j = ctr2 + sgn*(P2,Q2) or sgn*(R2,S2)
                    wxp = tF()
                    wyp = tF()
                    if sR == 0:
                        if sP > 0:
                            V.tensor_tensor(wxp, cx2, P2, op=OP.add)
                            V.tensor_tensor(wyp, cy2, Q2, op=OP.add)
                        else:
                            V.tensor_tensor(wxp, cx2, P2, op=OP.subtract)
                            V.tensor_tensor(wyp, cy2, Q2, op=OP.subtract)
                    else:
                        if sP > 0:
                            V.tensor_tensor(wxp, cx2, R2, op=OP.add)
                            V.tensor_tensor(wyp, cy2, S2, op=OP.add)
                        else:
                            V.tensor_tensor(wxp, cx2, R2, op=OP.subtract)
                            V.tensor_tensor(wyp, cy2, S2, op=OP.subtract)
                    # pw values for the 4 axes at this corner
                    pwj = {}
                    for ax in ("u1", "v1", "u2", "v2"):
                        o = tF()
                        dC = pj[(ax, "C")]
                        src = pj[(ax, "PQ2")] if sR == 0 else pj[(ax, "RS2")]
                        if sP > 0:
                            V.tensor_tensor(o, dC, src, op=OP.add)
                        else:
                            V.tensor_tensor(o, dC, src, op=OP.subtract)
                        pwj[ax] = o

                    def lin(ax):
                        o = view(t4F(), 4)
                        V.tensor_tensor(o, bc(pwj[ax], 4), pos[ax], op=OP.subtract)
                        return o

                    du1 = lin("u1")
                    dv1 = lin("v1")
                    du2 = lin("u2")
                    dv2 = lin("v2")
                    def aabs(x):
                        o = view(t4F(), 4)
                        S.activation(o, x, ACTF.Abs)
                        return o

                    adu1 = aabs(du1)
                    adv1 = aabs(dv1)
                    adu2 = aabs(du2)
                    adv2 = aabs(dv2)
                    h1d = view(t4F(), 4)
                    V.tensor_tensor(h1d, adu1, adv1, op=OP.add)
                    h2d = view(t4F(), 4)
                    V.tensor_tensor(h2d, adu2, adv2, op=OP.add)
                    h1p0 = view(t4F(), 4)
                    V.tensor_tensor(h1p0, bc(rat1, 4), adv1, op=OP.mult)
                    h1p1 = view(t4F(), 4)
                    V.tensor_tensor(h1p1, bc(irat1, 4), adu1, op=OP.mult)
                    h1p = view(t4F(), 4)
                    V.tensor_tensor(h1p, h1p0, h1p1, op=OP.add)
                    h2p0 = view(t4F(), 4)
                    V.tensor_tensor(h2p0, bc(rat2, 4), adv2, op=OP.mult)
                    h2p1 = view(t4F(), 4)
                    V.tensor_tensor(h2p1, bc(irat2, 4), adu2, op=OP.mult)
                    h2p = view(t4F(), 4)
                    V.tensor_tensor(h2p, h2p0, h2p1, op=OP.add)

                    dx = view(t4F(), 4)
                    V.tensor_tensor(dx, bc(wxp, 4), oxv, op=OP.subtract)
                    dy = view(t4F(), 4)
                    V.tensor_tensor(dy, bc(wyp, 4), oyv, op=OP.subtract)
                    dc0 = view(t4F(), 4)
                    V.tensor_tensor(dc0, dx, bc(cx2, 4), op=OP.mult)
                    dc1 = view(t4F(), 4)
                    V.tensor_tensor(dc1, dy, bc(cy2, 4), op=OP.mult)
                    dcv = view(t4F(), 4)
                    V.tensor_tensor(dcv, dc0, dc1, op=OP.add)
                    dp0 = view(t4F(), 4)
                    V.tensor_tensor(dp0, dx, bc(cy2, 4), op=OP.mult)
                    dp1 = view(t4F(), 4)
                    V.tensor_tensor(dp1, dy, bc(cx2, 4), op=OP.mult)
                    dcp = view(t4F(), 4)
                    V.tensor_tensor(dcp, dp0, dp1, op=OP.subtract)
                    sqx = view(t4F(), 4)
                    S.activation(sqx, dx, ACTF.Square)
                    sqy = view(t4F(), 4)
                    S.activation(sqy, dy, ACTF.Square)
                    dd = view(t4F(), 4)
                    V.tensor_tensor(dd, sqx, sqy, op=OP.add)

                    def rng(hA, hB, dcx):
                        ee1 = view(t4F(), 4)
                        V.tensor_tensor(ee1, dcx, hB, op=OP.add)
                        mm1 = view(t4F(), 4)
                        V.tensor_tensor(mm1, hA, ee1, op=OP.max)
                        ee2 = view(t4F(), 4)
                        V.tensor_tensor(ee2, hB, dcx, op=OP.subtract)
                        mm2 = view(t4F(), 4)
                        V.tensor_tensor(mm2, hA, ee2, op=OP.max)
                        o = view(t4F(), 4)
                        V.tensor_tensor(o, mm1, mm2, op=OP.add)
                        return o

                    rng_d = rng(h1d, h2d, dcv)
                    rng_p = rng(h1p, h2p, dcp)
                    ar = view(t4F(), 4)
                    V.tensor_tensor(ar, rng_d, rng_p, op=OP.mult)
                    dds = view(t4F(), 4)
                    V.tensor_scalar(dds, dd, 1e-30, None, op0=OP.max)
                    inv = view(t4F(), 4)
                    V.reciprocal(inv, dds)
                    ar2 = view(t4F(), 4)
                    V.tensor_tensor(ar2, ar, inv, op=OP.mult)
                    le = view(t4F(), 4)
                    V.tensor_scalar(le, dd, 1e-12, None, op0=OP.is_le)
                    if j == 0:
                        V.scalar_tensor_tensor(encminv, le, 1e18, ar2, op0=OP.mult, op1=OP.add)
                    else:
                        ar3 = view(t4F(), 4)
                        V.scalar_tensor_tensor(ar3, le, 1e18, ar2, op0=OP.mult, op1=OP.add)
                        V.tensor_tensor(encminv, encminv, ar3, op=OP.min)

                # --- rect-edge directions (4) ---
                red_dd = view(t4F(), 4)
                S.copy(red_dd[:, 0, :], hw1sq)
                S.copy(red_dd[:, 1, :], hl1sq)
                S.copy(red_dd[:, 2, :], hw2sq)
                S.copy(red_dd[:, 3, :], hl2sq)
                red_hop = view(t4F(), 4)
                S.copy(red_hop[:, 0, :], hwl1)
                S.copy(red_hop[:, 1, :], hwl1)
                S.copy(red_hop[:, 2, :], hwl2)
                S.copy(red_hop[:, 3, :], hwl2)
                red_hod = view(t4F(), 4)
                V.tensor_tensor(red_hod[:, 0, :], am_uu, am_vu, op=OP.add)
                V.tensor_tensor(red_hod[:, 1, :], am_uv, am_vv, op=OP.add)
                V.tensor_tensor(red_hod[:, 2, :], am_uu, am_uv, op=OP.add)
                V.tensor_tensor(red_hod[:, 3, :], am_vu, am_vv, op=OP.add)
                red_hpp = view(t4F(), 4)
                for k, (ra, ib, aa, ab) in enumerate((
                    (rat2, irat2, am_vu, am_uu),
                    (rat2, irat2, am_vv, am_uv),
                    (rat1, irat1, am_uv, am_uu),
                    (rat1, irat1, am_vv, am_vu),
                )):
                    ta = tF()
                    V.tensor_tensor(ta, ra, aa, op=OP.mult)
                    tb = tF()
                    V.tensor_tensor(tb, ib, ab, op=OP.mult)
                    V.tensor_tensor(red_hpp[:, k, :], ta, tb, op=OP.add)
                red_dc = view(t4F(), 4)
                for k, ax in enumerate(("u1", "v1", "u2", "v2")):
                    S.copy(red_dc[:, k, :], pj[(ax, "C")])
                red_dcp = view(t4F(), 4)
                t0_ = tF()
                V.tensor_tensor(t0_, A1, cy2, op=OP.mult)
                t1_ = tF()
                V.tensor_tensor(t1_, B1, cx2, op=OP.mult)
                V.tensor_tensor(red_dcp[:, 0, :], t0_, t1_, op=OP.subtract)
                t2_ = tF()
                V.tensor_tensor(t2_, nC1, cy2, op=OP.mult)
                t3_ = tF()
                V.tensor_tensor(t3_, D1, cx2, op=OP.mult)
                V.tensor_tensor(red_dcp[:, 1, :], t2_, t3_, op=OP.subtract)
                S.copy(red_dcp[:, 2, :], X_u)
                S.copy(red_dcp[:, 3, :], X_v)

                def rng4(hA, hB, dcx):
                    ee1 = view(t4F(), 4)
                    V.tensor_tensor(ee1, dcx, hB, op=OP.add)
                    mm1 = view(t4F(), 4)
                    V.tensor_tensor(mm1, hA, ee1, op=OP.max)
                    ee2 = view(t4F(), 4)
                    V.tensor_tensor(ee2, hB, dcx, op=OP.subtract)
                    mm2 = view(t4F(), 4)
                    V.tensor_tensor(mm2, hA, ee2, op=OP.max)
                    o = view(t4F(), 4)
                    V.tensor_tensor(o, mm1, mm2, op=OP.add)
                    return o

                r4d = rng4(red_dd, red_hod, red_dc)
                r4p = rng4(red_hop, red_hpp, red_dcp)
                ar4 = view(t4F(), 4)
                V.tensor_tensor(ar4, r4d, r4p, op=OP.mult)
                inv4 = view(t4F(), 4)
                V.reciprocal(inv4, red_dd)
                ar4b = view(t4F(), 4)
                V.tensor_tensor(ar4b, ar4, inv4, op=OP.mult)
                V.tensor_tensor(encminv, encminv, ar4b, op=OP.min)

                m2_ = view(t4F(), 4)[:, 0:2, :]
                V.tensor_tensor(m2_, encminv[:, 0:2, :], encminv[:, 2:4, :], op=OP.min)
                vc_min = tF()
                V.tensor_tensor(vc_min, m2_[:, 0, :], m2_[:, 1, :], op=OP.min)

                # ---------------- loss + reduce ----------------
                inter3d = tF()
                V.tensor_tensor(inter3d, inter2d, oz, op=OP.mult)
                usum = tF()
                V.tensor_tensor(usum, v1v, v2v, op=OP.add)
                union = tF()
                V.tensor_tensor(union, usum, inter3d, op=OP.subtract)
                um = tF()
                V.tensor_scalar(um, union, 1e-8, None, op0=OP.max)
                ru = tF()
                V.reciprocal(ru, um)
                iou3 = tF()
                V.tensor_tensor(iou3, inter3d, ru, op=OP.mult)
                vc = tF()
                V.tensor_tensor(vc, vc_min, zr, op=OP.mult)
                vcm = tF()
                V.tensor_scalar(vcm, vc, 1e-8, None, op0=OP.max)
                rvc = tF()
                V.reciprocal(rvc, vcm)
                tv_ = tF()
                V.tensor_tensor(tv_, union, rvc, op=OP.mult)
                sm = tF()
                V.tensor_tensor(sm, iou3, tv_, op=OP.add)
                giou = tF()
                V.tensor_scalar(giou, sm, -1.0, 2.0, op0=OP.mult, op1=OP.add)
                lm = tF()
                sum_acc = wp.tile([P, 1], FP, tag="sum_acc", name="sum_acc")[:]
                V.scalar_tensor_tensor(lm, giou, 1.0, mask, op0=OP.mult, op1=OP.mult,
                                       accum_out=sum_acc)
                cnt_dummy = tF()
                cnt_acc = wp.tile([P, 1], FP, tag="cnt_acc", name="cnt_acc")[:]
                V.tensor_scalar(cnt_dummy, mask, 1.0, 0.0, op0=OP.mult, op1=OP.add, accum_out=cnt_acc)

                outv = out_d.ap()
                nc.sync.dma_start(outv[:, 0:1], sum_acc)
                nc.sync.dma_start(outv[:, 1:2], cnt_acc)

    nc.compile()
    return nc


_NC = None


def kernel(pred: np.ndarray, target: np.ndarray, iou: np.ndarray) -> np.ndarray:
    global _NC
    if _NC is None:
        _NC = _build()
    in_maps = []
    for c in range(N_CORES):
        sl = slice(c * N_CORE, (c + 1) * N_CORE)
        in_maps.append({
            "pred": np.ascontiguousarray(pred[sl], dtype=np.float32),
            "target": np.ascontiguousarray(target[sl], dtype=np.float32),
            "iou": np.ascontiguousarray(iou[sl], dtype=np.float32),
        })
    res = bass_utils.run_bass_kernel_spmd(_NC, in_maps, core_ids=list(range(N_CORES)))
    tot = 0.0
    cnt = 0.0
    for r in res.results:
        tot += float(r["partials"][:, 0].astype(np.float64).sum())
        cnt += float(r["partials"][:, 1].astype(np.float64).sum())
    out = tot / max(cnt, 1.0) if cnt > 0 else 0.0
    return np.float32(out)



# revision 4
# speedup vs baseline: 2.5487x; 2.5487x over previous
"""Trainium2 Bass kernel for masked 3D-GIoU regression loss (262144 box pairs).

N_STREAMS independent single-core programs (default 2), each handling
N_TOTAL/N_STREAMS boxes as a sequence of chunks of 32768 boxes laid out as
128 partitions x 256 free elements. All geometry is elementwise plane ops:

  - BEV rotated-rect intersection via Liang-Barsky clipping + Green's theorem
    (no argsort; identical to the reference's angle-sort shoelace for
    non-degenerate inputs -- validated to 1e-14 per box in f64).
  - Smallest enclosing rectangle: min over 20 candidate directions
    (4 rect edge dirs + 16 corner cross-pairs). Equals the reference's
    28-pair min by the rotating-calipers theorem (diagonals are never hull
    edges of the union).
  - Work is balanced across the DVE / Activation / Pool engines
    (~245 / 153 / 153 G elem/s); per-chunk masked partial sums land in
    one output column pair, host sums and divides once.
"""

import sys
import numpy as np

if "/opt/trn_rl_repo" not in sys.path:
    sys.path.insert(0, "/opt/trn_rl_repo")

import concourse.bacc as bacc  # noqa: E402
import concourse.mybir as mybir  # noqa: E402
import concourse.tile as tile  # noqa: E402
from concourse import bass_utils  # noqa: E402
from concourse.alu_op_type import AluOpType as OP  # noqa: E402

N_STREAMS = 2
N_TOTAL = 262144
N_S = N_TOTAL // N_STREAMS   # boxes per stream
P = 128
F = 256                      # free elems per chunk
BLK = P * F                  # 32768 boxes per chunk
C = N_S // BLK               # chunks per stream
FP = mybir.dt.float32
ACTF = mybir.ActivationFunctionType
PI = float(np.pi)

# rotating temp-tag classes: tag -> (free elems, bufs)
_CLS = {
    "tF": (F, 10),
    "t4F": (4 * F, 11),
    "t8F": (8 * F, 5),
}


def _build():
    nc = bacc.Bacc("TRN2", target_bir_lowering=False, debug=False)
    pred_d = nc.dram_tensor("pred", [N_S, 7], FP, kind="ExternalInput")
    tgt_d = nc.dram_tensor("target", [N_S, 7], FP, kind="ExternalInput")
    iou_d = nc.dram_tensor("iou", [N_S], FP, kind="ExternalInput")
    out_d = nc.dram_tensor("partials", [P, 2 * C], FP, kind="ExternalOutput")

    V = nc.vector
    G = nc.gpsimd
    S = nc.scalar

    uid = [0]

    def mk(pool, cls):
        def t(_tag=None):
            uid[0] += 1
            fe, bufs = _CLS[cls]
            return pool.tile([P, fe], FP, tag=cls, bufs=bufs,
                             name=f"{cls}_{uid[0]}")[:]
        return t

    with tile.TileContext(nc) as tc:
        with tc.tile_pool(name="pers", bufs=1) as pers, \
             tc.tile_pool(name="work", bufs=1) as wp:
            tF = mk(wp, "tF")
            t4F = mk(wp, "t4F")
            t8F = mk(wp, "t8F")

            def PT(tag, shape=None):
                return pers.tile(shape or [P, F], FP, tag=tag, name=tag)[:]

            def view(ap, g):
                return ap.rearrange("p (g f) -> p g f", g=g)

            def bc(plane, g):
                return plane.rearrange("p (o f) -> p o f", o=1).broadcast_to([P, g, F])

            halfpi = PT("halfpi", [P, 1])
            V.memset(halfpi, PI / 2)

            consts = {}

            def cplane(c):
                if c not in consts:
                    t = pers.tile([P, 1], FP, tag=f"c_{len(consts)}",
                                  name=f"c_{len(consts)}")[:]
                    V.memset(t, float(c))
                    consts[c] = t
                return consts[c]

            def cb(c, like):
                t = cplane(c)
                shp = list(like.shape)
                pat = "p (" + " ".join(f"d{i}" for i in range(len(shp) - 1)) + ") -> p " + \
                      " ".join(f"d{i}" for i in range(len(shp) - 1))
                kw = {f"d{i}": 1 for i in range(len(shp) - 1)}
                return t.rearrange(pat, **kw).broadcast_to(shp)

            sumcnt = pers.tile([P, 2 * C], FP, tag="sumcnt", name="sumcnt")[:]

            for k in range(C):
                _emit_chunk(nc, tc, wp, pers, k, V, G, S, tF, t4F, t8F,
                            PT, view, bc, cb, halfpi, sumcnt,
                            pred_d, tgt_d, iou_d)

            nc.sync.dma_start(out_d.ap(), sumcnt)

    nc.compile()
    return nc


def _emit_chunk(nc, tc, wp, pers, k, V, G, S, tF, t4F, t8F,
                PT, view, bc, cb, halfpi, sumcnt,
                pred_d, tgt_d, iou_d):
    FPc = FP

    def relu(eng_s, out, in_):
        eng_s.activation(out, in_, ACTF.Relu)

    # ======== stage 1: DMA + per-box primitives ========
    predI = wp.tile([P, 7 * F], FPc, tag="predI", bufs=1, name=f"predI_{k}")[:]
    tgtI = wp.tile([P, 7 * F], FPc, tag="tgtI", bufs=1, name=f"tgtI_{k}")[:]
    iouP = wp.tile([P, F], FPc, tag="iouP", bufs=1, name=f"iouP_{k}")[:]
    sl = slice(k * BLK, (k + 1) * BLK)
    nc.sync.dma_start(predI, pred_d.ap()[sl, :].rearrange("(p f) c -> p (f c)", p=P))
    nc.sync.dma_start(tgtI, tgt_d.ap()[sl, :].rearrange("(p f) c -> p (f c)", p=P))
    nc.sync.dma_start(iouP, iou_d.ap()[sl].rearrange("(p f) -> p f", p=P))

    pv = predI.rearrange("p (f c) -> p c f", c=7)
    tv = tgtI.rearrange("p (f c) -> p c f", c=7)
    x1, y1, z1, w1, l1, h1, yaw1 = (pv[:, c, :] for c in range(7))
    x2, y2, z2, w2, l2, h2, yaw2 = (tv[:, c, :] for c in range(7))

    def sincos(eng, yaw, pfx):
        is_v = eng is V
        g1 = tF()
        V.tensor_scalar(g1, yaw, PI, None, op0=OP.is_gt)
        g2 = tF()
        V.tensor_scalar(g2, yaw, -PI, None, op0=OP.is_lt)
        adj = tF()
        eng.tensor_tensor(adj, g2, g1, op=OP.subtract)
        yr = tF()
        if is_v:
            eng.scalar_tensor_tensor(yr, adj, 2 * PI, yaw, op0=OP.mult, op1=OP.add)
        else:
            tmp = tF()
            eng.tensor_tensor(tmp, adj, cb(2 * PI, tmp), op=OP.mult)
            eng.tensor_tensor(yr, tmp, yaw, op=OP.add)
        sa = PT(pfx + "sa")
        S.activation(sa, yr, ACTF.Sin)
        g3 = tF()
        V.tensor_scalar(g3, yr, PI / 2, None, op0=OP.is_gt)
        yc = tF()
        if is_v:
            eng.scalar_tensor_tensor(yc, g3, -2 * PI, yr, op0=OP.mult, op1=OP.add)
        else:
            tmp2 = tF()
            eng.tensor_tensor(tmp2, g3, cb(-2 * PI, tmp2), op=OP.mult)
            eng.tensor_tensor(yc, tmp2, yr, op=OP.add)
        ca = PT(pfx + "ca")
        S.activation(ca, yc, ACTF.Sin, bias=halfpi)
        return sa, ca

    sa1, ca1 = sincos(V, yaw1, "t1")
    sa2, ca2 = sincos(G, yaw2, "t2")

    cx2 = PT("cx2")
    G.tensor_tensor(cx2, x2, x1, op=OP.subtract)
    cy2 = PT("cy2")
    G.tensor_tensor(cy2, y2, y1, op=OP.subtract)

    def halfaxes(eng, w, l, sa, ca, r):
        if eng is V:
            A = PT(f"A{r}")
            eng.scalar_tensor_tensor(A, w, 0.5, ca, op0=OP.mult, op1=OP.mult)
            B = PT(f"B{r}")
            eng.scalar_tensor_tensor(B, w, 0.5, sa, op0=OP.mult, op1=OP.mult)
            Cc = tF()
            eng.scalar_tensor_tensor(Cc, l, 0.5, sa, op0=OP.mult, op1=OP.mult)
            D = PT(f"D{r}")
            eng.scalar_tensor_tensor(D, l, 0.5, ca, op0=OP.mult, op1=OP.mult)
        else:
            hw = tF()
            S.mul(hw, w, 0.5)
            hl = tF()
            S.mul(hl, l, 0.5)
            A = PT(f"A{r}")
            eng.tensor_tensor(A, hw, ca, op=OP.mult)
            B = PT(f"B{r}")
            eng.tensor_tensor(B, hw, sa, op=OP.mult)
            Cc = tF()
            eng.tensor_tensor(Cc, hl, sa, op=OP.mult)
            D = PT(f"D{r}")
            eng.tensor_tensor(D, hl, ca, op=OP.mult)
        nC = PT(f"nC{r}")
        S.mul(nC, Cc, -1.0)
        Pp = PT(f"P{r}")
        eng.tensor_tensor(Pp, A, Cc, op=OP.subtract)
        Q = PT(f"Q{r}")
        eng.tensor_tensor(Q, B, D, op=OP.add)
        R = PT(f"R{r}")
        eng.tensor_tensor(R, A, Cc, op=OP.add)
        Ss = PT(f"S{r}")
        eng.tensor_tensor(Ss, B, D, op=OP.subtract)
        return A, B, D, nC, Pp, Q, R, Ss

    A1, B1, D1, nC1, P1, Q1, R1, S1 = halfaxes(V, w1, l1, sa1, ca1, 1)
    A2, B2, D2, nC2, P2, Q2, R2, S2 = halfaxes(G, w2, l2, sa2, ca2, 2)

    # z overlap / volumes / mask
    hh1 = tF()
    S.mul(hh1, h1, 0.5)
    hh2 = tF()
    S.mul(hh2, h2, 0.5)
    zmax1 = tF()
    G.tensor_tensor(zmax1, z1, hh1, op=OP.add)
    zmin1 = tF()
    G.tensor_tensor(zmin1, z1, hh1, op=OP.subtract)
    zmax2 = tF()
    G.tensor_tensor(zmax2, z2, hh2, op=OP.add)
    zmin2 = tF()
    G.tensor_tensor(zmin2, z2, hh2, op=OP.subtract)
    mn_hi = tF()
    V.tensor_tensor(mn_hi, zmax1, zmax2, op=OP.min)
    mx_lo = tF()
    V.tensor_tensor(mx_lo, zmin1, zmin2, op=OP.max)
    ozr = tF()
    G.tensor_tensor(ozr, mn_hi, mx_lo, op=OP.subtract)
    oz = PT("oz")
    relu(S, oz, ozr)
    mx_hi = tF()
    V.tensor_tensor(mx_hi, zmax1, zmax2, op=OP.max)
    mn_lo = tF()
    V.tensor_tensor(mn_lo, zmin1, zmin2, op=OP.min)
    zrr = tF()
    G.tensor_tensor(zrr, mx_hi, mn_lo, op=OP.subtract)
    zr = PT("zr")
    relu(S, zr, zrr)

    v1a = tF()
    G.tensor_tensor(v1a, w1, l1, op=OP.mult)
    v1v = PT("v1v")
    G.tensor_tensor(v1v, v1a, h1, op=OP.mult)
    v2a = tF()
    G.tensor_tensor(v2a, w2, l2, op=OP.mult)
    v2v = PT("v2v")
    G.tensor_tensor(v2v, v2a, h2, op=OP.mult)
    mask = PT("mask")
    V.tensor_scalar(mask, iouP, 0.55, None, op0=OP.is_ge)

    hw1sq = PT("hw1sq")
    V.scalar_tensor_tensor(hw1sq, w1, 0.25, w1, op0=OP.mult, op1=OP.mult)
    hl1sq = PT("hl1sq")
    V.scalar_tensor_tensor(hl1sq, l1, 0.25, l1, op0=OP.mult, op1=OP.mult)
    hw2sq = PT("hw2sq")
    V.scalar_tensor_tensor(hw2sq, w2, 0.25, w2, op0=OP.mult, op1=OP.mult)
    hl2sq = PT("hl2sq")
    V.scalar_tensor_tensor(hl2sq, l2, 0.25, l2, op0=OP.mult, op1=OP.mult)
    hwl1 = PT("hwl1")
    V.scalar_tensor_tensor(hwl1, w1, 0.25, l1, op0=OP.mult, op1=OP.mult)
    hwl2 = PT("hwl2")
    V.scalar_tensor_tensor(hwl2, w2, 0.25, l2, op0=OP.mult, op1=OP.mult)

    il1 = tF()
    V.reciprocal(il1, l1)
    rat1 = PT("rat1")
    V.tensor_tensor(rat1, w1, il1, op=OP.mult)
    iw1 = tF()
    V.reciprocal(iw1, w1)
    irat1 = PT("irat1")
    V.tensor_tensor(irat1, l1, iw1, op=OP.mult)
    il2 = tF()
    V.reciprocal(il2, l2)
    rat2 = PT("rat2")
    V.tensor_tensor(rat2, w2, il2, op=OP.mult)
    iw2 = tF()
    V.reciprocal(iw2, w2)
    irat2 = PT("irat2")
    V.tensor_tensor(irat2, l2, iw2, op=OP.mult)

    # ---- dots needed downstream (outputs persistent) ----
    def dot(eng, tag, ax, ay, bx, by):
        t0 = tF()
        eng.tensor_tensor(t0, ax, bx, op=OP.mult)
        t1 = tF()
        eng.tensor_tensor(t1, ay, by, op=OP.mult)
        o = PT(tag)
        eng.tensor_tensor(o, t0, t1, op=OP.add)
        return o

    def dott(eng, ax, ay, bx, by):
        t0 = tF()
        eng.tensor_tensor(t0, ax, bx, op=OP.mult)
        t1 = tF()
        eng.tensor_tensor(t1, ay, by, op=OP.mult)
        o = tF()
        eng.tensor_tensor(o, t0, t1, op=OP.add)
        return o

    def saferec(tag, m):
        g = tF()
        V.tensor_scalar(g, m, 0.0, None, op0=OP.is_ge)
        s2 = tF()
        V.tensor_scalar(s2, g, 2.0, 1.0, op0=OP.mult, op1=OP.subtract)
        am = PT(tag + "_am")
        S.activation(am, m, ACTF.Abs)
        amc = tF()
        V.tensor_scalar(amc, am, 1e-12, None, op0=OP.max)
        ms = tF()
        V.tensor_tensor(ms, s2, amc, op=OP.mult)
        o = PT(tag)
        V.reciprocal(o, ms)
        return o, am

    inv_uu, am_uu = saferec("inv_uu", dott(V, A2, B2, A1, B1))
    inv_uv, am_uv = saferec("inv_uv", dott(V, A2, B2, nC1, D1))
    inv_vu, am_vu = saferec("inv_vu", dott(V, nC2, D2, A1, B1))
    inv_vv, am_vv = saferec("inv_vv", dott(V, nC2, D2, nC1, D1))

    pj = {}
    for axname, axx, axy, eng in (
        ("u1", A1, B1, V),
        ("v1", nC1, D1, V),
        ("u2", A2, B2, G),
        ("v2", nC2, D2, G),
    ):
        for vec, vx, vy in (
            ("PQ1", P1, Q1),
            ("RS1", R1, S1),
            ("PQ2", P2, Q2),
            ("RS2", R2, S2),
            ("C", cx2, cy2),
        ):
            pj[(axname, vec)] = dot(eng, f"pj_{axname}_{vec}", axx, axy, vx, vy)

    # X_u = cx2*B2 - cy2*A2 ; X_v = cx2*D2 + cy2*C2
    xu0 = tF()
    G.tensor_tensor(xu0, cx2, B2, op=OP.mult)
    xu1 = tF()
    G.tensor_tensor(xu1, cy2, A2, op=OP.mult)
    X_u = PT("X_u")
    G.tensor_tensor(X_u, xu0, xu1, op=OP.subtract)
    xv0 = tF()
    G.tensor_tensor(xv0, cx2, D2, op=OP.mult)
    xv1 = tF()
    G.tensor_tensor(xv1, cy2, nC2, op=OP.mult)
    X_v = PT("X_vf")
    G.tensor_tensor(X_v, xv0, xv1, op=OP.subtract)  # cx2*D2 + cy2*C2

    # ======== stage 2: intersection ========
    def corner_su(eng, dPQ, dRS, dC, sign_off):
        outs = []
        for (src, sgn) in ((dPQ, 1), (dRS, -1), (dPQ, -1), (dRS, 1)):
            o = tF()
            if sign_off < 0:
                if sgn > 0:
                    eng.tensor_tensor(o, src, dC, op=OP.subtract)
                else:
                    if eng is V:
                        eng.scalar_tensor_tensor(o, src, -1.0, dC, op0=OP.mult, op1=OP.subtract)
                    else:
                        t = tF()
                        S.mul(t, src, -1.0)
                        eng.tensor_tensor(o, t, dC, op=OP.subtract)
            else:
                if sgn > 0:
                    eng.tensor_tensor(o, src, dC, op=OP.add)
                else:
                    eng.tensor_tensor(o, dC, src, op=OP.subtract)
            outs.append(o)
        return outs

    su1u = corner_su(V, pj[("u2", "PQ1")], pj[("u2", "RS1")], pj[("u2", "C")], -1)
    su1v = corner_su(V, pj[("v2", "PQ1")], pj[("v2", "RS1")], pj[("v2", "C")], -1)
    su2u = corner_su(G, pj[("u1", "PQ2")], pj[("u1", "RS2")], pj[("u1", "C")], +1)
    su2v = corner_su(G, pj[("v1", "PQ2")], pj[("v1", "RS2")], pj[("v1", "C")], +1)

    def emit_pass(eng, su_by_axis, h_by_axis, inv_by_edge_axis):
        is_v = eng is V
        suA, suB = su_by_axis
        hA, hB = h_by_axis
        su_s = t8F()
        suv = su_s.rearrange("p (e a f) -> p e a f", e=4, a=2)
        inv_s = t8F()
        invv = inv_s.rearrange("p (e a f) -> p e a f", e=4, a=2)
        h_s = t4F()
        hv = view(h_s, 4)
        S.copy(hv[:, 0, :], hA)
        S.copy(hv[:, 1, :], hB)
        for e in range(4):
            S.copy(suv[:, e, 0, :], suA[e])
            S.copy(suv[:, e, 1, :], suB[e])
            for a in range(2):
                ip, cf = inv_by_edge_axis[e][a]
                # for POOL: store NEGATED inv so r1 = (su+h)*(-inv)
                S.mul(invv[:, e, a, :], ip, cf if is_v else -cf)
        hb = (h_s[:, 0:2 * F]
              .rearrange("p (o a f) -> p o a f", o=1, a=2)
              .broadcast_to([P, 4, 2, F]))
        sus = su_s.rearrange("p (e a f) -> p e a f", e=4, a=2)
        a1 = t8F()
        if is_v:
            eng.scalar_tensor_tensor(a1.rearrange("p (e a f) -> p e a f", e=4, a=2),
                                     sus, -1.0, hb, op0=OP.mult, op1=OP.subtract)
        else:
            # a1 = su + h ; combined with negated inv gives same r1
            eng.tensor_tensor(a1.rearrange("p (e a f) -> p e a f", e=4, a=2),
                              sus, hb, op=OP.add)
        a2 = t8F()
        if is_v:
            eng.tensor_tensor(a2.rearrange("p (e a f) -> p e a f", e=4, a=2),
                              hb, sus, op=OP.subtract)
        else:
            # r2 = (h-su)*inv = (su-h)*(-inv); inv strip holds -inv
            eng.tensor_tensor(a2.rearrange("p (e a f) -> p e a f", e=4, a=2),
                              sus, hb, op=OP.subtract)
        r1 = t8F()
        eng.tensor_tensor(r1, a1, inv_s, op=OP.mult)
        r2 = t8F()
        eng.tensor_tensor(r2, a2, inv_s, op=OP.mult)
        lo = t8F()
        V.tensor_tensor(lo, r1, r2, op=OP.min)
        hi = t8F()
        V.tensor_tensor(hi, r1, r2, op=OP.max)
        lov = lo.rearrange("p (e a f) -> p e a f", e=4, a=2)
        hiv = hi.rearrange("p (e a f) -> p e a f", e=4, a=2)
        t0p = t4F()
        V.tensor_tensor(view(t0p, 4), lov[:, :, 0, :], lov[:, :, 1, :], op=OP.max)
        t0 = t4F()
        relu(S, t0, t0p)
        t1p = t4F()
        V.tensor_tensor(view(t1p, 4), hiv[:, :, 0, :], hiv[:, :, 1, :], op=OP.min)
        t1 = t4F()
        V.tensor_scalar(t1, t1p, 1.0, None, op0=OP.min)
        dt = t4F()
        eng.tensor_tensor(dt, t1, t0, op=OP.subtract)
        dtc = t4F()
        relu(S, dtc, dt)
        return dtc

    inv1 = [
        [(inv_uu, -0.5), (inv_vu, -0.5)],
        [(inv_uv, -0.5), (inv_vv, -0.5)],
        [(inv_uu, 0.5), (inv_vu, 0.5)],
        [(inv_uv, 0.5), (inv_vv, 0.5)],
    ]
    dt1 = emit_pass(V, (su1u, su1v), (hw2sq, hl2sq), inv1)
    inv2 = [
        [(inv_uu, -0.5), (inv_uv, -0.5)],
        [(inv_vu, -0.5), (inv_vv, -0.5)],
        [(inv_uu, 0.5), (inv_uv, 0.5)],
        [(inv_vu, 0.5), (inv_vv, 0.5)],
    ]
    dt2 = emit_pass(G, (su2u, su2v), (hw1sq, hl1sq), inv2)

    dt1v = view(dt1, 4)
    sa_ = t4F()
    V.tensor_tensor(view(sa_, 4)[:, 0:2, :], dt1v[:, 0:2, :], dt1v[:, 2:4, :], op=OP.add)
    sav = view(sa_, 4)
    sum1 = tF()
    V.tensor_tensor(sum1, sav[:, 0, :], sav[:, 1, :], op=OP.add)
    contrib1 = tF()
    V.tensor_tensor(contrib1, sum1, hwl1, op=OP.mult)

    dt2v = view(dt2, 4)
    sb_ = t4F()
    G.tensor_tensor(view(sb_, 4)[:, 0:2, :], dt2v[:, 0:2, :], dt2v[:, 2:4, :], op=OP.add)
    sbv = view(sb_, 4)
    sum2 = tF()
    G.tensor_tensor(sum2, sbv[:, 0, :], sbv[:, 1, :], op=OP.add)
    base2 = tF()
    G.tensor_tensor(base2, sum2, hwl2, op=OP.mult)
    d20 = tF()
    G.tensor_tensor(d20, dt2v[:, 2, :], dt2v[:, 0, :], op=OP.subtract)
    d31 = tF()
    G.tensor_tensor(d31, dt2v[:, 3, :], dt2v[:, 1, :], op=OP.subtract)
    tXu = tF()
    G.tensor_tensor(tXu, d20, X_u, op=OP.mult)
    tXv = tF()
    G.tensor_tensor(tXv, d31, X_v, op=OP.mult)
    c2s = tF()
    G.tensor_tensor(c2s, base2, tXu, op=OP.add)
    c2t = tF()
    G.tensor_tensor(c2t, c2s, tXv, op=OP.add)
    isum = tF()
    V.tensor_tensor(isum, contrib1, c2t, op=OP.add)
    inter2d = PT("inter2d")
    V.scalar_tensor_tensor(inter2d, isum, -1.0, isum, op0=OP.mult, op1=OP.max)

    # ======== stage 3: enclosing rectangle ========
    ox = wp.tile([P, 4 * F], FPc, tag="ox", name=f"ox_{k}")[:]
    oxv = view(ox, 4)
    S.copy(oxv[:, 0, :], P1)
    S.mul(oxv[:, 1, :], R1, -1.0)
    S.mul(oxv[:, 2, :], P1, -1.0)
    S.copy(oxv[:, 3, :], R1)
    oy = wp.tile([P, 4 * F], FPc, tag="oy", name=f"oy_{k}")[:]
    oyv = view(oy, 4)
    S.copy(oyv[:, 0, :], Q1)
    S.mul(oyv[:, 1, :], S1, -1.0)
    S.mul(oyv[:, 2, :], Q1, -1.0)
    S.copy(oyv[:, 3, :], S1)
    pos = {}
    for ax in ("u1", "v1", "u2", "v2"):
        st = wp.tile([P, 4 * F], FPc, tag=f"po_{ax}", name=f"po_{ax}_{k}")[:]
        sv = view(st, 4)
        dPQ1 = pj[(ax, "PQ1")]
        dRS1 = pj[(ax, "RS1")]
        S.copy(sv[:, 0, :], dPQ1)
        S.mul(sv[:, 1, :], dRS1, -1.0)
        S.mul(sv[:, 2, :], dPQ1, -1.0)
        S.copy(sv[:, 3, :], dRS1)
        pos[ax] = sv

    encmin = wp.tile([P, 4 * F], FPc, tag="encmin", name=f"encmin_{k}")[:]
    encminv = view(encmin, 4)

    # per-corner-j group of 4 cross directions; alternate V / G per group
    for j in range(4):
        E = V if j % 2 == 0 else G
        sP, sR = ((1, 0), (-1, 1), (-1, 0), (1, 1))[j]
        # rect2 corner j = ctr2 + sgn*(P2,Q2) or sgn*(R2,S2)
        wxp = tF()
        wyp = tF()
        if sR == 0:
            if sP > 0:
                E.tensor_tensor(wxp, cx2, P2, op=OP.add)
                E.tensor_tensor(wyp, cy2, Q2, op=OP.add)
            else:
                E.tensor_tensor(wxp, cx2, P2, op=OP.subtract)
                E.tensor_tensor(wyp, cy2, Q2, op=OP.subtract)
        else:
            if sP > 0:
                E.tensor_tensor(wxp, cx2, R2, op=OP.add)
                E.tensor_tensor(wyp, cy2, S2, op=OP.add)
            else:
                E.tensor_tensor(wxp, cx2, R2, op=OP.subtract)
                E.tensor_tensor(wyp, cy2, S2, op=OP.subtract)
        # pw values for the 4 axes at this corner
        pwj = {}
        for ax in ("u1", "v1", "u2", "v2"):
            o = tF()
            dC = pj[(ax, "C")]
            src = pj[(ax, "PQ2")] if sR == 0 else pj[(ax, "RS2")]
            if sP > 0:
                E.tensor_tensor(o, dC, src, op=OP.add)
            else:
                E.tensor_tensor(o, dC, src, op=OP.subtract)
            pwj[ax] = o

        def lin(ax):
            o = view(t4F(), 4)
            E.tensor_tensor(o, bc(pwj[ax], 4), pos[ax], op=OP.subtract)
            return o

        du1 = lin("u1")
        dv1 = lin("v1")
        du2 = lin("u2")
        dv2 = lin("v2")

        def aabs(x):
            o = view(t4F(), 4)
            S.activation(o, x, ACTF.Abs)
            return o

        adu1 = aabs(du1)
        adv1 = aabs(dv1)
        adu2 = aabs(du2)
        adv2 = aabs(dv2)
        h1d = view(t4F(), 4)
        E.tensor_tensor(h1d, adu1, adv1, op=OP.add)
        h2d = view(t4F(), 4)
        E.tensor_tensor(h2d, adu2, adv2, op=OP.add)
        h1p0 = view(t4F(), 4)
        E.tensor_tensor(h1p0, bc(rat1, 4), adv1, op=OP.mult)
        h1p1 = view(t4F(), 4)
        E.tensor_tensor(h1p1, bc(irat1, 4), adu1, op=OP.mult)
        h1p = view(t4F(), 4)
        E.tensor_tensor(h1p, h1p0, h1p1, op=OP.add)
        h2p0 = view(t4F(), 4)
        E.tensor_tensor(h2p0, bc(rat2, 4), adv2, op=OP.mult)
        h2p1 = view(t4F(), 4)
        E.tensor_tensor(h2p1, bc(irat2, 4), adu2, op=OP.mult)
        h2p = view(t4F(), 4)
        E.tensor_tensor(h2p, h2p0, h2p1, op=OP.add)

        dx = view(t4F(), 4)
        E.tensor_tensor(dx, bc(wxp, 4), oxv, op=OP.subtract)
        dy = view(t4F(), 4)
        E.tensor_tensor(dy, bc(wyp, 4), oyv, op=OP.subtract)
        dc0 = view(t4F(), 4)
        E.tensor_tensor(dc0, dx, bc(cx2, 4), op=OP.mult)
        dc1 = view(t4F(), 4)
        E.tensor_tensor(dc1, dy, bc(cy2, 4), op=OP.mult)
        dcv = view(t4F(), 4)
        E.tensor_tensor(dcv, dc0, dc1, op=OP.add)
        dp0 = view(t4F(), 4)
        E.tensor_tensor(dp0, dx, bc(cy2, 4), op=OP.mult)
        dp1 = view(t4F(), 4)
        E.tensor_tensor(dp1, dy, bc(cx2, 4), op=OP.mult)
        dcp = view(t4F(), 4)
        E.tensor_tensor(dcp, dp0, dp1, op=OP.subtract)
        sqx = view(t4F(), 4)
        S.activation(sqx, dx, ACTF.Square)
        sqy = view(t4F(), 4)
        S.activation(sqy, dy, ACTF.Square)
        dd = view(t4F(), 4)
        E.tensor_tensor(dd, sqx, sqy, op=OP.add)

        def rng(hA, hB, dcx):
            ee1 = view(t4F(), 4)
            E.tensor_tensor(ee1, dcx, hB, op=OP.add)
            mm1 = view(t4F(), 4)
            V.tensor_tensor(mm1, hA, ee1, op=OP.max)
            ee2 = view(t4F(), 4)
            E.tensor_tensor(ee2, hB, dcx, op=OP.subtract)
            mm2 = view(t4F(), 4)
            V.tensor_tensor(mm2, hA, ee2, op=OP.max)
            o = view(t4F(), 4)
            E.tensor_tensor(o, mm1, mm2, op=OP.add)
            return o

        rng_d = rng(h1d, h2d, dcv)
        rng_p = rng(h1p, h2p, dcp)
        ar = view(t4F(), 4)
        E.tensor_tensor(ar, rng_d, rng_p, op=OP.mult)
        dds = view(t4F(), 4)
        V.tensor_scalar(dds, dd, 1e-30, None, op0=OP.max)
        inv = view(t4F(), 4)
        V.reciprocal(inv, dds)
        ar2 = view(t4F(), 4)
        E.tensor_tensor(ar2, ar, inv, op=OP.mult)
        le = view(t4F(), 4)
        V.tensor_scalar(le, dd, 1e-12, None, op0=OP.is_le)
        if j == 0:
            V.scalar_tensor_tensor(encminv, le, 1e18, ar2, op0=OP.mult, op1=OP.add)
        else:
            ar3 = view(t4F(), 4)
            V.scalar_tensor_tensor(ar3, le, 1e18, ar2, op0=OP.mult, op1=OP.add)
            V.tensor_tensor(encminv, encminv, ar3, op=OP.min)

    # --- rect-edge directions (4) ---
    red_dd = view(t4F(), 4)
    S.copy(red_dd[:, 0, :], hw1sq)
    S.copy(red_dd[:, 1, :], hl1sq)
    S.copy(red_dd[:, 2, :], hw2sq)
    S.copy(red_dd[:, 3, :], hl2sq)
    red_hop = view(t4F(), 4)
    S.copy(red_hop[:, 0, :], hwl1)
    S.copy(red_hop[:, 1, :], hwl1)
    S.copy(red_hop[:, 2, :], hwl2)
    S.copy(red_hop[:, 3, :], hwl2)
    red_hod = view(t4F(), 4)
    G.tensor_tensor(red_hod[:, 0, :], am_uu, am_vu, op=OP.add)
    G.tensor_tensor(red_hod[:, 1, :], am_uv, am_vv, op=OP.add)
    G.tensor_tensor(red_hod[:, 2, :], am_uu, am_uv, op=OP.add)
    G.tensor_tensor(red_hod[:, 3, :], am_vu, am_vv, op=OP.add)
    red_hpp = view(t4F(), 4)
    for kk, (ra, ib, aa, ab) in enumerate((
        (rat2, irat2, am_vu, am_uu),
        (rat2, irat2, am_vv, am_uv),
        (rat1, irat1, am_uv, am_uu),
        (rat1, irat1, am_vv, am_vu),
    )):
        ta = tF()
        G.tensor_tensor(ta, ra, aa, op=OP.mult)
        tb = tF()
        G.tensor_tensor(tb, ib, ab, op=OP.mult)
        G.tensor_tensor(red_hpp[:, kk, :], ta, tb, op=OP.add)
    red_dc = view(t4F(), 4)
    for kk, ax in enumerate(("u1", "v1", "u2", "v2")):
        S.copy(red_dc[:, kk, :], pj[(ax, "C")])
    red_dcp = view(t4F(), 4)
    t0_ = tF()
    G.tensor_tensor(t0_, A1, cy2, op=OP.mult)
    t1_ = tF()
    G.tensor_tensor(t1_, B1, cx2, op=OP.mult)
    G.tensor_tensor(red_dcp[:, 0, :], t0_, t1_, op=OP.subtract)
    t2_ = tF()
    G.tensor_tensor(t2_, nC1, cy2, op=OP.mult)
    t3_ = tF()
    G.tensor_tensor(t3_, D1, cx2, op=OP.mult)
    G.tensor_tensor(red_dcp[:, 1, :], t2_, t3_, op=OP.subtract)
    S.copy(red_dcp[:, 2, :], X_u)
    S.copy(red_dcp[:, 3, :], X_v)

    def rng4(hA, hB, dcx):
        ee1 = view(t4F(), 4)
        G.tensor_tensor(ee1, dcx, hB, op=OP.add)
        mm1 = view(t4F(), 4)
        V.tensor_tensor(mm1, hA, ee1, op=OP.max)
        ee2 = view(t4F(), 4)
        G.tensor_tensor(ee2, hB, dcx, op=OP.subtract)
        mm2 = view(t4F(), 4)
        V.tensor_tensor(mm2, hA, ee2, op=OP.max)
        o = view(t4F(), 4)
        G.tensor_tensor(o, mm1, mm2, op=OP.add)
        return o

    r4d = rng4(red_dd, red_hod, red_dc)
    r4p = rng4(red_hop, red_hpp, red_dcp)
    ar4 = view(t4F(), 4)
    G.tensor_tensor(ar4, r4d, r4p, op=OP.mult)
    inv4 = view(t4F(), 4)
    V.reciprocal(inv4, red_dd)
    ar4b = view(t4F(), 4)
    G.tensor_tensor(ar4b, ar4, inv4, op=OP.mult)
    V.tensor_tensor(encminv, encminv, ar4b, op=OP.min)

    m2_ = view(t4F(), 4)[:, 0:2, :]
    V.tensor_tensor(m2_, encminv[:, 0:2, :], encminv[:, 2:4, :], op=OP.min)
    vc_min = tF()
    V.tensor_tensor(vc_min, m2_[:, 0, :], m2_[:, 1, :], op=OP.min)

    # ======== stage 4: loss + reduce ========
    inter3d = tF()
    V.tensor_tensor(inter3d, inter2d, oz, op=OP.mult)
    usum = tF()
    G.tensor_tensor(usum, v1v, v2v, op=OP.add)
    union = tF()
    V.tensor_tensor(union, usum, inter3d, op=OP.subtract)
    um = tF()
    V.tensor_scalar(um, union, 1e-8, None, op0=OP.max)
    ru = tF()
    V.reciprocal(ru, um)
    iou3 = tF()
    V.tensor_tensor(iou3, inter3d, ru, op=OP.mult)
    vc = tF()
    G.tensor_tensor(vc, vc_min, zr, op=OP.mult)
    vcm = tF()
    V.tensor_scalar(vcm, vc, 1e-8, None, op0=OP.max)
    rvc = tF()
    V.reciprocal(rvc, vcm)
    tv_ = tF()
    G.tensor_tensor(tv_, union, rvc, op=OP.mult)
    sm = tF()
    V.tensor_tensor(sm, iou3, tv_, op=OP.add)
    giou = tF()
    V.tensor_scalar(giou, sm, -1.0, 2.0, op0=OP.mult, op1=OP.add)
    lm = tF()
    V.scalar_tensor_tensor(lm, giou, 1.0, mask, op0=OP.mult, op1=OP.mult,
                           accum_out=sumcnt[:, 2 * k:2 * k + 1])
    cnt_dummy = tF()
    V.tensor_scalar(cnt_dummy, mask, 1.0, 0.0, op0=OP.mult, op1=OP.add,
                    accum_out=sumcnt[:, 2 * k + 1:2 * k + 2])


_NC = None


def _get_nc():
    global _NC
    if _NC is None:
        _NC = _build()
    return _NC


def kernel(pred: np.ndarray, target: np.ndarray, iou: np.ndarray) -> np.ndarray:
    nc = _get_nc()
    in_maps = []
    for c in range(N_STREAMS):
        sl = slice(c * N_S, (c + 1) * N_S)
        in_maps.append({
            "pred": np.ascontiguousarray(pred[sl], dtype=np.float32),
            "target": np.ascontiguousarray(target[sl], dtype=np.float32),
            "iou": np.ascontiguousarray(iou[sl], dtype=np.float32),
        })
    res = bass_utils.run_bass_kernel_spmd(nc, in_maps, core_ids=list(range(N_STREAMS)))
    tot = 0.0
    cnt = 0.0
    for r in res.results:
        pr = r["partials"].reshape(P, C, 2)
        tot += float(pr[:, :, 0].astype(np.float64).sum())
        cnt += float(pr[:, :, 1].astype(np.float64).sum())
    out = tot / max(cnt, 1.0) if cnt > 0 else 0.0
    return np.float32(out)


# revision 10
# speedup vs baseline: 4.5070x; 1.7683x over previous
"""Trainium2 Bass kernel for masked 3D-GIoU regression loss (262144 box pairs).

N_STREAMS independent single-core programs (default 2), each handling
N_TOTAL/N_STREAMS boxes as a sequence of chunks of 32768 boxes laid out as
128 partitions x 256 free elements. All geometry is elementwise plane ops:

  - BEV rotated-rect intersection via Liang-Barsky clipping + Green's theorem
    (no argsort; identical to the reference's angle-sort shoelace for
    non-degenerate inputs -- validated to 1e-14 per box in f64).
  - Smallest enclosing rectangle: min over 20 candidate directions
    (4 rect edge dirs + 16 corner cross-pairs). Equals the reference's
    28-pair min by the rotating-calipers theorem (diagonals are never hull
    edges of the union).
  - Work is balanced across the DVE / Activation / Pool engines
    (~245 / 153 / 153 G elem/s); per-chunk masked partial sums land in
    one output column pair, host sums and divides once.
"""

import sys
import numpy as np

if "/opt/trn_rl_repo" not in sys.path:
    sys.path.insert(0, "/opt/trn_rl_repo")

import concourse.bacc as bacc  # noqa: E402
import concourse.mybir as mybir  # noqa: E402
import concourse.tile as tile  # noqa: E402
from concourse import bass_utils  # noqa: E402
from concourse.alu_op_type import AluOpType as OP  # noqa: E402

N_STREAMS = 4
DEBUG = False
N_TOTAL = 262144
N_S = N_TOTAL // N_STREAMS   # boxes per stream
P = 128
F = 256                      # free elems per chunk
BLK = P * F                  # 32768 boxes per chunk
C = N_S // BLK               # chunks per stream
FP = mybir.dt.float32
BF = mybir.dt.bfloat16
ACTF = mybir.ActivationFunctionType
PI = float(np.pi)

# rotating temp-tag classes: tag -> (free elems, bufs, dtype)
_CLS = {
    "tF": (F, 10, mybir.dt.float32),
    "t4F": (4 * F, 8, mybir.dt.float32),
    "bF": (F, 12, mybir.dt.bfloat16),
    "b4F": (4 * F, 11, mybir.dt.bfloat16),
    "b8F": (8 * F, 5, mybir.dt.bfloat16),
}


def _build():
    nc = bacc.Bacc("TRN2", target_bir_lowering=False, debug=False)
    pred_d = nc.dram_tensor("pred", [N_S, 7], FP, kind="ExternalInput")
    tgt_d = nc.dram_tensor("target", [N_S, 7], FP, kind="ExternalInput")
    iou_d = nc.dram_tensor("iou", [N_S], FP, kind="ExternalInput")
    out_d = nc.dram_tensor("partials", [P, 2 * C], FP, kind="ExternalOutput")
    dbg_d = None
    if DEBUG:
        dbg_d = nc.dram_tensor("dbg", [P, 2 * C * F], FP, kind="ExternalOutput")

    V = nc.vector
    G = nc.gpsimd
    S = nc.scalar

    uid = [0]

    def mk(pool, cls):
        def t(_tag=None):
            uid[0] += 1
            fe, bufs, dt = _CLS[cls]
            return pool.tile([P, fe], dt, tag=cls, bufs=bufs,
                             name=f"{cls}_{uid[0]}")[:]
        return t

    with tile.TileContext(nc) as tc:
        with tc.tile_pool(name="pers", bufs=1) as pers, \
             tc.tile_pool(name="work", bufs=1) as wp:
            tF = mk(wp, "tF")
            t4F = mk(wp, "t4F")
            bF = mk(wp, "bF")
            b4F = mk(wp, "b4F")
            b8F = mk(wp, "b8F")

            def PT(tag, shape=None, dt=FP):
                return pers.tile(shape or [P, F], dt, tag=tag, name=tag)[:]

            def view(ap, g):
                return ap.rearrange("p (g f) -> p g f", g=g)

            def bc(plane, g):
                return plane.rearrange("p (o f) -> p o f", o=1).broadcast_to([P, g, F])

            halfpi = PT("halfpi", [P, 1])
            V.memset(halfpi, PI / 2)

            consts = {}

            def cplane(c):
                if c not in consts:
                    t = pers.tile([P, 1], FP, tag=f"c_{len(consts)}",
                                  name=f"c_{len(consts)}")[:]
                    V.memset(t, float(c))
                    consts[c] = t
                return consts[c]

            def cb(c, like):
                t = cplane(c)
                shp = list(like.shape)
                pat = "p (" + " ".join(f"d{i}" for i in range(len(shp) - 1)) + ") -> p " + \
                      " ".join(f"d{i}" for i in range(len(shp) - 1))
                kw = {f"d{i}": 1 for i in range(len(shp) - 1)}
                return t.rearrange(pat, **kw).broadcast_to(shp)

            sumcnt = pers.tile([P, 2 * C], FP, tag="sumcnt", name="sumcnt")[:]

            for k in range(C):
                _emit_chunk(nc, tc, wp, pers, k, V, G, S, tF, t4F, bF, b4F, b8F,
                            PT, view, bc, cb, halfpi, sumcnt,
                            pred_d, tgt_d, iou_d, dbg_d)

            nc.sync.dma_start(out_d.ap(), sumcnt)

    nc.compile()
    return nc


def _emit_chunk(nc, tc, wp, pers, k, V, G, S, tF, t4F, bF, b4F, b8F,
                PT, view, bc, cb, halfpi, sumcnt,
                pred_d, tgt_d, iou_d, dbg_d=None):
    FPc = FP

    def relu(eng_s, out, in_):
        eng_s.activation(out, in_, ACTF.Relu)

    # ======== stage 1: DMA + per-box primitives ========
    predI = wp.tile([P, 7 * F], FPc, tag="predI", bufs=1, name=f"predI_{k}")[:]
    tgtI = wp.tile([P, 7 * F], FPc, tag="tgtI", bufs=1, name=f"tgtI_{k}")[:]
    iouP = wp.tile([P, F], FPc, tag="iouP", bufs=1, name=f"iouP_{k}")[:]
    sl = slice(k * BLK, (k + 1) * BLK)
    nc.sync.dma_start(predI, pred_d.ap()[sl, :].rearrange("(p f) c -> p (f c)", p=P))
    nc.sync.dma_start(tgtI, tgt_d.ap()[sl, :].rearrange("(p f) c -> p (f c)", p=P))
    nc.sync.dma_start(iouP, iou_d.ap()[sl].rearrange("(p f) -> p f", p=P))

    pv = predI.rearrange("p (f c) -> p c f", c=7)
    tv = tgtI.rearrange("p (f c) -> p c f", c=7)
    x1, y1, z1, w1, l1, h1, yaw1 = (pv[:, c, :] for c in range(7))
    x2, y2, z2, w2, l2, h2, yaw2 = (tv[:, c, :] for c in range(7))

    def sincos(eng, yaw, pfx):
        is_v = eng is V
        g1 = tF()
        V.tensor_scalar(g1, yaw, PI, None, op0=OP.is_gt)
        g2 = tF()
        V.tensor_scalar(g2, yaw, -PI, None, op0=OP.is_lt)
        adj = tF()
        eng.tensor_tensor(adj, g2, g1, op=OP.subtract)
        yr = tF()
        if is_v:
            eng.scalar_tensor_tensor(yr, adj, 2 * PI, yaw, op0=OP.mult, op1=OP.add)
        else:
            tmp = tF()
            eng.tensor_tensor(tmp, adj, cb(2 * PI, tmp), op=OP.mult)
            eng.tensor_tensor(yr, tmp, yaw, op=OP.add)
        sa = PT(pfx + "sa")
        S.activation(sa, yr, ACTF.Sin)
        g3 = tF()
        V.tensor_scalar(g3, yr, PI / 2, None, op0=OP.is_gt)
        yc = tF()
        if is_v:
            eng.scalar_tensor_tensor(yc, g3, -2 * PI, yr, op0=OP.mult, op1=OP.add)
        else:
            tmp2 = tF()
            eng.tensor_tensor(tmp2, g3, cb(-2 * PI, tmp2), op=OP.mult)
            eng.tensor_tensor(yc, tmp2, yr, op=OP.add)
        ca = PT(pfx + "ca")
        S.activation(ca, yc, ACTF.Sin, bias=halfpi)
        return sa, ca

    sa1, ca1 = sincos(V, yaw1, "t1")
    sa2, ca2 = sincos(G, yaw2, "t2")

    cx2 = PT("cx2")
    G.tensor_tensor(cx2, x2, x1, op=OP.subtract)
    cy2 = PT("cy2")
    G.tensor_tensor(cy2, y2, y1, op=OP.subtract)

    def halfaxes(eng, w, l, sa, ca, r):
        if eng is V:
            A = PT(f"A{r}")
            eng.scalar_tensor_tensor(A, w, 0.5, ca, op0=OP.mult, op1=OP.mult)
            B = PT(f"B{r}")
            eng.scalar_tensor_tensor(B, w, 0.5, sa, op0=OP.mult, op1=OP.mult)
            Cc = tF()
            eng.scalar_tensor_tensor(Cc, l, 0.5, sa, op0=OP.mult, op1=OP.mult)
            D = PT(f"D{r}")
            eng.scalar_tensor_tensor(D, l, 0.5, ca, op0=OP.mult, op1=OP.mult)
        else:
            hw = tF()
            S.mul(hw, w, 0.5)
            hl = tF()
            S.mul(hl, l, 0.5)
            A = PT(f"A{r}")
            eng.tensor_tensor(A, hw, ca, op=OP.mult)
            B = PT(f"B{r}")
            eng.tensor_tensor(B, hw, sa, op=OP.mult)
            Cc = tF()
            eng.tensor_tensor(Cc, hl, sa, op=OP.mult)
            D = PT(f"D{r}")
            eng.tensor_tensor(D, hl, ca, op=OP.mult)
        nC = PT(f"nC{r}")
        S.mul(nC, Cc, -1.0)
        Pp = PT(f"P{r}")
        eng.tensor_tensor(Pp, A, Cc, op=OP.subtract)
        Q = PT(f"Q{r}")
        eng.tensor_tensor(Q, B, D, op=OP.add)
        R = PT(f"R{r}")
        eng.tensor_tensor(R, A, Cc, op=OP.add)
        Ss = PT(f"S{r}")
        eng.tensor_tensor(Ss, B, D, op=OP.subtract)
        return A, B, D, nC, Pp, Q, R, Ss

    A1, B1, D1, nC1, P1, Q1, R1, S1 = halfaxes(V, w1, l1, sa1, ca1, 1)
    A2, B2, D2, nC2, P2, Q2, R2, S2 = halfaxes(G, w2, l2, sa2, ca2, 2)

    # z overlap / volumes / mask
    hh1 = tF()
    S.mul(hh1, h1, 0.5)
    hh2 = tF()
    S.mul(hh2, h2, 0.5)
    zmax1 = tF()
    G.tensor_tensor(zmax1, z1, hh1, op=OP.add)
    zmin1 = tF()
    G.tensor_tensor(zmin1, z1, hh1, op=OP.subtract)
    zmax2 = tF()
    G.tensor_tensor(zmax2, z2, hh2, op=OP.add)
    zmin2 = tF()
    G.tensor_tensor(zmin2, z2, hh2, op=OP.subtract)
    mn_hi = tF()
    V.tensor_tensor(mn_hi, zmax1, zmax2, op=OP.min)
    mx_lo = tF()
    V.tensor_tensor(mx_lo, zmin1, zmin2, op=OP.max)
    ozr = tF()
    G.tensor_tensor(ozr, mn_hi, mx_lo, op=OP.subtract)
    oz = PT("oz")
    relu(S, oz, ozr)
    mx_hi = tF()
    V.tensor_tensor(mx_hi, zmax1, zmax2, op=OP.max)
    mn_lo = tF()
    V.tensor_tensor(mn_lo, zmin1, zmin2, op=OP.min)
    zrr = tF()
    G.tensor_tensor(zrr, mx_hi, mn_lo, op=OP.subtract)
    zr = PT("zr")
    relu(S, zr, zrr)

    v1a = tF()
    G.tensor_tensor(v1a, w1, l1, op=OP.mult)
    v1v = PT("v1v")
    G.tensor_tensor(v1v, v1a, h1, op=OP.mult)
    v2a = tF()
    G.tensor_tensor(v2a, w2, l2, op=OP.mult)
    v2v = PT("v2v")
    G.tensor_tensor(v2v, v2a, h2, op=OP.mult)
    mask = PT("mask")
    V.tensor_scalar(mask, iouP, 0.55, None, op0=OP.is_ge)

    hw1sq = PT("hw1sq")
    V.scalar_tensor_tensor(hw1sq, w1, 0.25, w1, op0=OP.mult, op1=OP.mult)
    hl1sq = PT("hl1sq")
    V.scalar_tensor_tensor(hl1sq, l1, 0.25, l1, op0=OP.mult, op1=OP.mult)
    hw2sq = PT("hw2sq")
    V.scalar_tensor_tensor(hw2sq, w2, 0.25, w2, op0=OP.mult, op1=OP.mult)
    hl2sq = PT("hl2sq")
    V.scalar_tensor_tensor(hl2sq, l2, 0.25, l2, op0=OP.mult, op1=OP.mult)
    hwl1 = PT("hwl1")
    V.scalar_tensor_tensor(hwl1, w1, 0.25, l1, op0=OP.mult, op1=OP.mult)
    hwl2 = PT("hwl2")
    V.scalar_tensor_tensor(hwl2, w2, 0.25, l2, op0=OP.mult, op1=OP.mult)

    il1 = tF()
    V.reciprocal(il1, l1)
    rat1 = PT("rat1")
    V.tensor_tensor(rat1, w1, il1, op=OP.mult)
    iw1 = tF()
    V.reciprocal(iw1, w1)
    irat1 = PT("irat1")
    V.tensor_tensor(irat1, l1, iw1, op=OP.mult)
    il2 = tF()
    V.reciprocal(il2, l2)
    rat2 = PT("rat2")
    V.tensor_tensor(rat2, w2, il2, op=OP.mult)
    iw2 = tF()
    V.reciprocal(iw2, w2)
    irat2 = PT("irat2")
    V.tensor_tensor(irat2, l2, iw2, op=OP.mult)

    # ---- bf16 working copies of the shared primitives (Act converts) ----
    def BPT(tag, srcv):
        o = PT("b" + tag, dt=BF)
        S.copy(o, srcv)
        return o

    bA1 = BPT("A1", A1); bB1 = BPT("B1", B1); bD1 = BPT("D1", D1); bnC1 = BPT("nC1", nC1)
    bP1 = BPT("P1", P1); bQ1 = BPT("Q1", Q1); bR1 = BPT("R1", R1); bS1 = BPT("S1", S1)
    bA2 = BPT("A2", A2); bB2 = BPT("B2", B2); bD2 = BPT("D2", D2); bnC2 = BPT("nC2", nC2)
    bP2 = BPT("P2", P2); bQ2 = BPT("Q2", Q2); bR2 = BPT("R2", R2); bS2 = BPT("S2", S2)
    bcx2 = BPT("cx2", cx2); bcy2 = BPT("cy2", cy2)
    brat1 = BPT("rat1", rat1); birat1 = BPT("irat1", irat1)
    brat2 = BPT("rat2", rat2); birat2 = BPT("irat2", irat2)

    # ---- dots needed downstream (outputs persistent) ----
    def dot(eng, tag, ax, ay, bx, by):
        t0 = tF()
        eng.tensor_tensor(t0, ax, bx, op=OP.mult)
        t1 = tF()
        eng.tensor_tensor(t1, ay, by, op=OP.mult)
        o = PT(tag)
        eng.tensor_tensor(o, t0, t1, op=OP.add)
        return o

    def dott(eng, ax, ay, bx, by):
        t0 = tF()
        eng.tensor_tensor(t0, ax, bx, op=OP.mult)
        t1 = tF()
        eng.tensor_tensor(t1, ay, by, op=OP.mult)
        o = tF()
        eng.tensor_tensor(o, t0, t1, op=OP.add)
        return o

    def saferec(tag, m):
        g = tF()
        V.tensor_scalar(g, m, 0.0, None, op0=OP.is_ge)
        s2 = tF()
        V.tensor_scalar(s2, g, 2.0, 1.0, op0=OP.mult, op1=OP.subtract)
        am = PT(tag + "_am")
        S.activation(am, m, ACTF.Abs)
        amc = tF()
        V.tensor_scalar(amc, am, 1e-12, None, op0=OP.max)
        ms = tF()
        V.tensor_tensor(ms, s2, amc, op=OP.mult)
        o = PT(tag)
        V.reciprocal(o, ms)
        return o, am

    inv_uu, am_uu = saferec("inv_uu", dott(V, A2, B2, A1, B1))
    inv_uv, am_uv = saferec("inv_uv", dott(V, A2, B2, nC1, D1))
    inv_vu, am_vu = saferec("inv_vu", dott(V, nC2, D2, A1, B1))
    inv_vv, am_vv = saferec("inv_vv", dott(V, nC2, D2, nC1, D1))

    def bdot(eng, tag, ax, ay, bx, by):
        t0 = bF()
        eng.tensor_tensor(t0, ax, bx, op=OP.mult)
        t1 = bF()
        eng.tensor_tensor(t1, ay, by, op=OP.mult)
        o = PT(tag, dt=BF)
        eng.tensor_tensor(o, t0, t1, op=OP.add)
        return o

    pj = {}
    for axname, axx, axy, eng in (
        ("u1", bA1, bB1, V),
        ("v1", bnC1, bD1, V),
        ("u2", bA2, bB2, G),
        ("v2", bnC2, bD2, G),
    ):
        for vec, vx, vy in (
            ("PQ1", bP1, bQ1),
            ("RS1", bR1, bS1),
            ("PQ2", bP2, bQ2),
            ("RS2", bR2, bS2),
            ("C", bcx2, bcy2),
        ):
            pj[(axname, vec)] = bdot(eng, f"pj_{axname}_{vec}", axx, axy, vx, vy)

    # X_u = cx2*B2 - cy2*A2 ; X_v = cx2*D2 + cy2*C2
    xu0 = tF()
    G.tensor_tensor(xu0, cx2, B2, op=OP.mult)
    xu1 = tF()
    G.tensor_tensor(xu1, cy2, A2, op=OP.mult)
    X_u = PT("X_u")
    G.tensor_tensor(X_u, xu0, xu1, op=OP.subtract)
    xv0 = tF()
    G.tensor_tensor(xv0, cx2, D2, op=OP.mult)
    xv1 = tF()
    G.tensor_tensor(xv1, cy2, nC2, op=OP.mult)
    X_v = PT("X_vf")
    G.tensor_tensor(X_v, xv0, xv1, op=OP.subtract)  # cx2*D2 + cy2*C2

    # ======== stage 2: intersection ========
    def corner_su(eng, dPQ, dRS, dC, sign_off):
        outs = []
        for (src, sgn) in ((dPQ, 1), (dRS, -1), (dPQ, -1), (dRS, 1)):
            o = bF()
            if sign_off < 0:
                if sgn > 0:
                    eng.tensor_tensor(o, src, dC, op=OP.subtract)
                else:
                    if eng is V:
                        eng.scalar_tensor_tensor(o, src, -1.0, dC, op0=OP.mult, op1=OP.subtract)
                    else:
                        t = bF()
                        S.mul(t, src, -1.0)
                        eng.tensor_tensor(o, t, dC, op=OP.subtract)
            else:
                if sgn > 0:
                    eng.tensor_tensor(o, src, dC, op=OP.add)
                else:
                    eng.tensor_tensor(o, dC, src, op=OP.subtract)
            outs.append(o)
        return outs

    su1u = corner_su(V, pj[("u2", "PQ1")], pj[("u2", "RS1")], pj[("u2", "C")], -1)
    su1v = corner_su(V, pj[("v2", "PQ1")], pj[("v2", "RS1")], pj[("v2", "C")], -1)
    su2u = corner_su(G, pj[("u1", "PQ2")], pj[("u1", "RS2")], pj[("u1", "C")], +1)
    su2v = corner_su(G, pj[("v1", "PQ2")], pj[("v1", "RS2")], pj[("v1", "C")], +1)

    def emit_pass(eng, su_by_axis, h_by_axis, inv_by_edge_axis):
        is_v = eng is V
        suA, suB = su_by_axis
        hA, hB = h_by_axis
        su_s = b8F()
        suv = su_s.rearrange("p (e a f) -> p e a f", e=4, a=2)
        inv_s = b8F()
        invv = inv_s.rearrange("p (e a f) -> p e a f", e=4, a=2)
        h_s = b4F()
        hv = view(h_s, 4)
        S.copy(hv[:, 0, :], hA)
        S.copy(hv[:, 1, :], hB)
        for e in range(4):
            S.copy(suv[:, e, 0, :], suA[e])
            S.copy(suv[:, e, 1, :], suB[e])
            for a in range(2):
                ip, cf = inv_by_edge_axis[e][a]
                # for POOL: store NEGATED inv so r1 = (su+h)*(-inv)
                S.mul(invv[:, e, a, :], ip, cf if is_v else -cf)
        hb = (h_s[:, 0:2 * F]
              .rearrange("p (o a f) -> p o a f", o=1, a=2)
              .broadcast_to([P, 4, 2, F]))
        sus = su_s.rearrange("p (e a f) -> p e a f", e=4, a=2)
        a1 = b8F()
        if is_v:
            eng.scalar_tensor_tensor(a1.rearrange("p (e a f) -> p e a f", e=4, a=2),
                                     sus, -1.0, hb, op0=OP.mult, op1=OP.subtract)
        else:
            # a1 = su + h ; combined with negated inv gives same r1
            eng.tensor_tensor(a1.rearrange("p (e a f) -> p e a f", e=4, a=2),
                              sus, hb, op=OP.add)
        a2 = b8F()
        if is_v:
            eng.tensor_tensor(a2.rearrange("p (e a f) -> p e a f", e=4, a=2),
                              hb, sus, op=OP.subtract)
        else:
            # r2 = (h-su)*inv = (su-h)*(-inv); inv strip holds -inv
            eng.tensor_tensor(a2.rearrange("p (e a f) -> p e a f", e=4, a=2),
                              sus, hb, op=OP.subtract)
        r1 = b8F()
        eng.tensor_tensor(r1, a1, inv_s, op=OP.mult)
        r2 = b8F()
        eng.tensor_tensor(r2, a2, inv_s, op=OP.mult)
        lo = b8F()
        V.tensor_tensor(lo, r1, r2, op=OP.min)
        hi = b8F()
        V.tensor_tensor(hi, r1, r2, op=OP.max)
        lov = lo.rearrange("p (e a f) -> p e a f", e=4, a=2)
        hiv = hi.rearrange("p (e a f) -> p e a f", e=4, a=2)
        t0p = b4F()
        V.tensor_tensor(view(t0p, 4), lov[:, :, 0, :], lov[:, :, 1, :], op=OP.max)
        t0 = b4F()
        relu(S, t0, t0p)
        t1p = b4F()
        V.tensor_tensor(view(t1p, 4), hiv[:, :, 0, :], hiv[:, :, 1, :], op=OP.min)
        t1 = b4F()
        V.tensor_scalar(t1, t1p, 1.0, None, op0=OP.min)
        dt = b4F()
        eng.tensor_tensor(dt, t1, t0, op=OP.subtract)
        dtc = b4F()
        relu(S, dtc, dt)
        return dtc

    inv1 = [
        [(inv_uu, -0.5), (inv_vu, -0.5)],
        [(inv_uv, -0.5), (inv_vv, -0.5)],
        [(inv_uu, 0.5), (inv_vu, 0.5)],
        [(inv_uv, 0.5), (inv_vv, 0.5)],
    ]
    dt1 = emit_pass(V, (su1u, su1v), (hw2sq, hl2sq), inv1)
    inv2 = [
        [(inv_uu, -0.5), (inv_uv, -0.5)],
        [(inv_vu, -0.5), (inv_vv, -0.5)],
        [(inv_uu, 0.5), (inv_uv, 0.5)],
        [(inv_vu, 0.5), (inv_vv, 0.5)],
    ]
    dt2 = emit_pass(G, (su2u, su2v), (hw1sq, hl1sq), inv2)

    dt1v = view(dt1, 4)
    sa_ = b4F()
    V.tensor_tensor(view(sa_, 4)[:, 0:2, :], dt1v[:, 0:2, :], dt1v[:, 2:4, :], op=OP.add)
    sav = view(sa_, 4)
    sum1 = bF()
    V.tensor_tensor(sum1, sav[:, 0, :], sav[:, 1, :], op=OP.add)
    contrib1 = tF()
    V.tensor_tensor(contrib1, sum1, hwl1, op=OP.mult)

    dt2v = view(dt2, 4)
    sb_ = b4F()
    G.tensor_tensor(view(sb_, 4)[:, 0:2, :], dt2v[:, 0:2, :], dt2v[:, 2:4, :], op=OP.add)
    sbv = view(sb_, 4)
    sum2 = bF()
    G.tensor_tensor(sum2, sbv[:, 0, :], sbv[:, 1, :], op=OP.add)
    base2 = tF()
    G.tensor_tensor(base2, sum2, hwl2, op=OP.mult)
    d20 = bF()
    G.tensor_tensor(d20, dt2v[:, 2, :], dt2v[:, 0, :], op=OP.subtract)
    d31 = bF()
    G.tensor_tensor(d31, dt2v[:, 3, :], dt2v[:, 1, :], op=OP.subtract)
    tXu = tF()
    G.tensor_tensor(tXu, d20, X_u, op=OP.mult)
    tXv = tF()
    G.tensor_tensor(tXv, d31, X_v, op=OP.mult)
    c2s = tF()
    G.tensor_tensor(c2s, base2, tXu, op=OP.add)
    c2t = tF()
    G.tensor_tensor(c2t, c2s, tXv, op=OP.add)
    isum = tF()
    V.tensor_tensor(isum, contrib1, c2t, op=OP.add)
    inter2d = PT("inter2d")
    V.scalar_tensor_tensor(inter2d, isum, -1.0, isum, op0=OP.mult, op1=OP.max)

    # ======== stage 3: enclosing rectangle ========
    ox = wp.tile([P, 4 * F], BF, tag="ox", name=f"ox_{k}")[:]
    oxv = view(ox, 4)
    S.copy(oxv[:, 0, :], P1)
    S.mul(oxv[:, 1, :], R1, -1.0)
    S.mul(oxv[:, 2, :], P1, -1.0)
    S.copy(oxv[:, 3, :], R1)
    oy = wp.tile([P, 4 * F], BF, tag="oy", name=f"oy_{k}")[:]
    oyv = view(oy, 4)
    S.copy(oyv[:, 0, :], Q1)
    S.mul(oyv[:, 1, :], S1, -1.0)
    S.mul(oyv[:, 2, :], Q1, -1.0)
    S.copy(oyv[:, 3, :], S1)
    pos = {}
    for ax in ("u1", "v1", "u2", "v2"):
        st = wp.tile([P, 4 * F], BF, tag=f"po_{ax}", name=f"po_{ax}_{k}")[:]
        sv = view(st, 4)
        dPQ1 = pj[(ax, "PQ1")]
        dRS1 = pj[(ax, "RS1")]
        S.copy(sv[:, 0, :], dPQ1)
        S.mul(sv[:, 1, :], dRS1, -1.0)
        S.mul(sv[:, 2, :], dPQ1, -1.0)
        S.copy(sv[:, 3, :], dRS1)
        pos[ax] = sv

    encmin = wp.tile([P, 4 * F], FPc, tag="encmin", name=f"encmin_{k}")[:]
    encminv = view(encmin, 4)

    # per-corner-j group of 4 cross directions; alternate V / G per group
    for j in range(4):
        E = V if j < 3 else G
        sP, sR = ((1, 0), (-1, 1), (-1, 0), (1, 1))[j]
        # rect2 corner j = ctr2 + sgn*(P2,Q2) or sgn*(R2,S2)
        wxp = bF()
        wyp = bF()
        if sR == 0:
            if sP > 0:
                E.tensor_tensor(wxp, bcx2, bP2, op=OP.add)
                E.tensor_tensor(wyp, bcy2, bQ2, op=OP.add)
            else:
                E.tensor_tensor(wxp, bcx2, bP2, op=OP.subtract)
                E.tensor_tensor(wyp, bcy2, bQ2, op=OP.subtract)
        else:
            if sP > 0:
                E.tensor_tensor(wxp, bcx2, bR2, op=OP.add)
                E.tensor_tensor(wyp, bcy2, bS2, op=OP.add)
            else:
                E.tensor_tensor(wxp, bcx2, bR2, op=OP.subtract)
                E.tensor_tensor(wyp, bcy2, bS2, op=OP.subtract)
        # pw values for the 4 axes at this corner
        pwj = {}
        for ax in ("u1", "v1", "u2", "v2"):
            o = bF()
            dC = pj[(ax, "C")]
            src = pj[(ax, "PQ2")] if sR == 0 else pj[(ax, "RS2")]
            if sP > 0:
                E.tensor_tensor(o, dC, src, op=OP.add)
            else:
                E.tensor_tensor(o, dC, src, op=OP.subtract)
            pwj[ax] = o

        def lin(ax):
            o = view(b4F(), 4)
            E.tensor_tensor(o, bc(pwj[ax], 4), pos[ax], op=OP.subtract)
            return o

        du1 = lin("u1")
        dv1 = lin("v1")
        du2 = lin("u2")
        dv2 = lin("v2")

        def aabs(x):
            o = view(b4F(), 4)
            S.activation(o, x, ACTF.Abs)
            return o

        adu1 = aabs(du1)
        adv1 = aabs(dv1)
        adu2 = aabs(du2)
        adv2 = aabs(dv2)
        h1d = view(b4F(), 4)
        E.tensor_tensor(h1d, adu1, adv1, op=OP.add)
        h2d = view(b4F(), 4)
        E.tensor_tensor(h2d, adu2, adv2, op=OP.add)
        h1p0 = view(b4F(), 4)
        E.tensor_tensor(h1p0, bc(brat1, 4), adv1, op=OP.mult)
        h1p1 = view(b4F(), 4)
        E.tensor_tensor(h1p1, bc(birat1, 4), adu1, op=OP.mult)
        h1p = view(b4F(), 4)
        E.tensor_tensor(h1p, h1p0, h1p1, op=OP.add)
        h2p0 = view(b4F(), 4)
        E.tensor_tensor(h2p0, bc(brat2, 4), adv2, op=OP.mult)
        h2p1 = view(b4F(), 4)
        E.tensor_tensor(h2p1, bc(birat2, 4), adu2, op=OP.mult)
        h2p = view(b4F(), 4)
        E.tensor_tensor(h2p, h2p0, h2p1, op=OP.add)

        ED = V if j == 3 else E
        dx = view(b4F(), 4)
        ED.tensor_tensor(dx, bc(wxp, 4), oxv, op=OP.subtract)
        dy = view(b4F(), 4)
        ED.tensor_tensor(dy, bc(wyp, 4), oyv, op=OP.subtract)
        dc0 = view(b4F(), 4)
        ED.tensor_tensor(dc0, dx, bc(bcx2, 4), op=OP.mult)
        dc1 = view(b4F(), 4)
        ED.tensor_tensor(dc1, dy, bc(bcy2, 4), op=OP.mult)
        dcv = view(b4F(), 4)
        ED.tensor_tensor(dcv, dc0, dc1, op=OP.add)
        dp0 = view(b4F(), 4)
        ED.tensor_tensor(dp0, dx, bc(bcy2, 4), op=OP.mult)
        dp1 = view(b4F(), 4)
        ED.tensor_tensor(dp1, dy, bc(bcx2, 4), op=OP.mult)
        dcp = view(b4F(), 4)
        ED.tensor_tensor(dcp, dp0, dp1, op=OP.subtract)
        sqx = view(t4F(), 4)
        S.activation(sqx, dx, ACTF.Square)
        sqy = view(t4F(), 4)
        S.activation(sqy, dy, ACTF.Square)
        dd = view(t4F(), 4)
        E.tensor_tensor(dd, sqx, sqy, op=OP.add)

        def rng(hA, hB, dcx):
            ee1 = view(b4F(), 4)
            E.tensor_tensor(ee1, dcx, hB, op=OP.add)
            mm1 = view(b4F(), 4)
            V.tensor_tensor(mm1, hA, ee1, op=OP.max)
            ee2 = view(b4F(), 4)
            E.tensor_tensor(ee2, hB, dcx, op=OP.subtract)
            mm2 = view(b4F(), 4)
            V.tensor_tensor(mm2, hA, ee2, op=OP.max)
            o = view(b4F(), 4)
            E.tensor_tensor(o, mm1, mm2, op=OP.add)
            return o

        rng_d = rng(h1d, h2d, dcv)
        rng_p = rng(h1p, h2p, dcp)
        ar = view(t4F(), 4)
        E.tensor_tensor(ar, rng_d, rng_p, op=OP.mult)
        dds = view(t4F(), 4)
        V.tensor_scalar(dds, dd, 1e-30, None, op0=OP.max)
        inv = view(t4F(), 4)
        V.reciprocal(inv, dds)
        ar2 = view(t4F(), 4)
        E.tensor_tensor(ar2, ar, inv, op=OP.mult)
        le = view(t4F(), 4)
        V.tensor_scalar(le, dd, 0.25, None, op0=OP.is_le)
        if j == 0:
            V.scalar_tensor_tensor(encminv, le, 1e18, ar2, op0=OP.mult, op1=OP.add)
        else:
            ar3 = view(t4F(), 4)
            V.scalar_tensor_tensor(ar3, le, 1e18, ar2, op0=OP.mult, op1=OP.add)
            V.tensor_tensor(encminv, encminv, ar3, op=OP.min)

    # --- rect-edge directions (4) ---
    red_dd = view(t4F(), 4)
    S.copy(red_dd[:, 0, :], hw1sq)
    S.copy(red_dd[:, 1, :], hl1sq)
    S.copy(red_dd[:, 2, :], hw2sq)
    S.copy(red_dd[:, 3, :], hl2sq)
    red_hop = view(t4F(), 4)
    S.copy(red_hop[:, 0, :], hwl1)
    S.copy(red_hop[:, 1, :], hwl1)
    S.copy(red_hop[:, 2, :], hwl2)
    S.copy(red_hop[:, 3, :], hwl2)
    red_hod = view(t4F(), 4)
    G.tensor_tensor(red_hod[:, 0, :], am_uu, am_vu, op=OP.add)
    G.tensor_tensor(red_hod[:, 1, :], am_uv, am_vv, op=OP.add)
    G.tensor_tensor(red_hod[:, 2, :], am_uu, am_uv, op=OP.add)
    G.tensor_tensor(red_hod[:, 3, :], am_vu, am_vv, op=OP.add)
    red_hpp = view(t4F(), 4)
    for kk, (ra, ib, aa, ab) in enumerate((
        (rat2, irat2, am_vu, am_uu),
        (rat2, irat2, am_vv, am_uv),
        (rat1, irat1, am_uv, am_uu),
        (rat1, irat1, am_vv, am_vu),
    )):
        ta = tF()
        G.tensor_tensor(ta, ra, aa, op=OP.mult)
        tb = tF()
        G.tensor_tensor(tb, ib, ab, op=OP.mult)
        G.tensor_tensor(red_hpp[:, kk, :], ta, tb, op=OP.add)
    red_dc = view(t4F(), 4)
    for kk, ax in enumerate(("u1", "v1", "u2", "v2")):
        S.copy(red_dc[:, kk, :], pj[(ax, "C")])
    red_dcp = view(t4F(), 4)
    t0_ = tF()
    G.tensor_tensor(t0_, A1, cy2, op=OP.mult)
    t1_ = tF()
    G.tensor_tensor(t1_, B1, cx2, op=OP.mult)
    G.tensor_tensor(red_dcp[:, 0, :], t0_, t1_, op=OP.subtract)
    t2_ = tF()
    G.tensor_tensor(t2_, nC1, cy2, op=OP.mult)
    t3_ = tF()
    G.tensor_tensor(t3_, D1, cx2, op=OP.mult)
    G.tensor_tensor(red_dcp[:, 1, :], t2_, t3_, op=OP.subtract)
    S.copy(red_dcp[:, 2, :], X_u)
    S.copy(red_dcp[:, 3, :], X_v)

    def rng4(hA, hB, dcx):
        ee1 = view(t4F(), 4)
        G.tensor_tensor(ee1, dcx, hB, op=OP.add)
        mm1 = view(t4F(), 4)
        V.tensor_tensor(mm1, hA, ee1, op=OP.max)
        ee2 = view(t4F(), 4)
        G.tensor_tensor(ee2, hB, dcx, op=OP.subtract)
        mm2 = view(t4F(), 4)
        V.tensor_tensor(mm2, hA, ee2, op=OP.max)
        o = view(t4F(), 4)
        G.tensor_tensor(o, mm1, mm2, op=OP.add)
        return o

    r4d = rng4(red_dd, red_hod, red_dc)
    r4p = rng4(red_hop, red_hpp, red_dcp)
    ar4 = view(t4F(), 4)
    G.tensor_tensor(ar4, r4d, r4p, op=OP.mult)
    inv4 = view(t4F(), 4)
    V.reciprocal(inv4, red_dd)
    ar4b = view(t4F(), 4)
    G.tensor_tensor(ar4b, ar4, inv4, op=OP.mult)
    V.tensor_tensor(encminv, encminv, ar4b, op=OP.min)

    m2_ = view(t4F(), 4)[:, 0:2, :]
    V.tensor_tensor(m2_, encminv[:, 0:2, :], encminv[:, 2:4, :], op=OP.min)
    vc_min = tF()
    V.tensor_tensor(vc_min, m2_[:, 0, :], m2_[:, 1, :], op=OP.min)

    if dbg_d is not None:
        nc.sync.dma_start(dbg_d.ap()[:, (2 * k) * F:(2 * k + 1) * F], inter2d)
        nc.sync.dma_start(dbg_d.ap()[:, (2 * k + 1) * F:(2 * k + 2) * F], vc_min)

    # ======== stage 4: loss + reduce ========
    inter3d = tF()
    V.tensor_tensor(inter3d, inter2d, oz, op=OP.mult)
    usum = tF()
    G.tensor_tensor(usum, v1v, v2v, op=OP.add)
    union = tF()
    V.tensor_tensor(union, usum, inter3d, op=OP.subtract)
    um = tF()
    V.tensor_scalar(um, union, 1e-8, None, op0=OP.max)
    ru = tF()
    V.reciprocal(ru, um)
    iou3 = tF()
    V.tensor_tensor(iou3, inter3d, ru, op=OP.mult)
    vc = tF()
    G.tensor_tensor(vc, vc_min, zr, op=OP.mult)
    vcm = tF()
    V.tensor_scalar(vcm, vc, 1e-8, None, op0=OP.max)
    rvc = tF()
    V.reciprocal(rvc, vcm)
    tv_ = tF()
    G.tensor_tensor(tv_, union, rvc, op=OP.mult)
    sm = tF()
    V.tensor_tensor(sm, iou3, tv_, op=OP.add)
    giou = tF()
    V.tensor_scalar(giou, sm, -1.0, 2.0, op0=OP.mult, op1=OP.add)
    lm = tF()
    V.scalar_tensor_tensor(lm, giou, 1.0, mask, op0=OP.mult, op1=OP.mult,
                           accum_out=sumcnt[:, 2 * k:2 * k + 1])
    cnt_dummy = tF()
    V.tensor_scalar(cnt_dummy, mask, 1.0, 0.0, op0=OP.mult, op1=OP.add,
                    accum_out=sumcnt[:, 2 * k + 1:2 * k + 2])


_NC = None


def _get_nc():
    global _NC
    if _NC is None:
        _NC = _build()
    return _NC


def kernel(pred: np.ndarray, target: np.ndarray, iou: np.ndarray) -> np.ndarray:
    nc = _get_nc()
    in_maps = []
    for c in range(N_STREAMS):
        sl = slice(c * N_S, (c + 1) * N_S)
        in_maps.append({
            "pred": np.ascontiguousarray(pred[sl], dtype=np.float32),
            "target": np.ascontiguousarray(target[sl], dtype=np.float32),
            "iou": np.ascontiguousarray(iou[sl], dtype=np.float32),
        })
    res = bass_utils.run_bass_kernel_spmd(nc, in_maps, core_ids=list(range(N_STREAMS)))
    tot = 0.0
    cnt = 0.0
    for r in res.results:
        pr = r["partials"].reshape(P, C, 2)
        tot += float(pr[:, :, 0].astype(np.float64).sum())
        cnt += float(pr[:, :, 1].astype(np.float64).sum())
    out = tot / max(cnt, 1.0) if cnt > 0 else 0.0
    return np.float32(out)


# revision 15
# speedup vs baseline: 8.5824x; 1.9042x over previous
"""Trainium2 Bass kernel for masked 3D-GIoU regression loss (262144 box pairs).

N_STREAMS independent single-core programs (default 2), each handling
N_TOTAL/N_STREAMS boxes as a sequence of chunks of 32768 boxes laid out as
128 partitions x 256 free elements. All geometry is elementwise plane ops:

  - BEV rotated-rect intersection via Liang-Barsky clipping + Green's theorem
    (no argsort; identical to the reference's angle-sort shoelace for
    non-degenerate inputs -- validated to 1e-14 per box in f64).
  - Smallest enclosing rectangle: min over 20 candidate directions
    (4 rect edge dirs + 16 corner cross-pairs). Equals the reference's
    28-pair min by the rotating-calipers theorem (diagonals are never hull
    edges of the union).
  - Work is balanced across the DVE / Activation / Pool engines
    (~245 / 153 / 153 G elem/s); per-chunk masked partial sums land in
    one output column pair, host sums and divides once.
"""

import sys
import numpy as np

if "/opt/trn_rl_repo" not in sys.path:
    sys.path.insert(0, "/opt/trn_rl_repo")

import concourse.bacc as bacc  # noqa: E402
import concourse.mybir as mybir  # noqa: E402
import concourse.tile as tile  # noqa: E402
from concourse import bass_utils  # noqa: E402
from concourse.alu_op_type import AluOpType as OP  # noqa: E402

N_STREAMS = 4
DEBUG = False
N_TOTAL = 262144
N_S = N_TOTAL // N_STREAMS   # boxes per stream
P = 128
F = 256                      # free elems per chunk
BLK = P * F                  # 32768 boxes per chunk
C = N_S // BLK               # chunks per stream
FP = mybir.dt.float32
BF = mybir.dt.bfloat16
ACTF = mybir.ActivationFunctionType
PI = float(np.pi)

# rotating temp-tag classes: tag -> (free elems, bufs, dtype)
_CLS = {
    "tF": (F, 12, mybir.dt.float32),
    "t4F": (4 * F, 10, mybir.dt.float32),
    "bF": (F, 16, mybir.dt.bfloat16),
    "b4F": (4 * F, 11, mybir.dt.bfloat16),
    "b8F": (8 * F, 5, mybir.dt.bfloat16),
}


def _build():
    nc = bacc.Bacc("TRN2", target_bir_lowering=False, debug=False)
    pred_d = nc.dram_tensor("pred", [N_S, 7], FP, kind="ExternalInput")
    tgt_d = nc.dram_tensor("target", [N_S, 7], FP, kind="ExternalInput")
    iou_d = nc.dram_tensor("iou", [N_S], FP, kind="ExternalInput")
    out_d = nc.dram_tensor("partials", [P, 2 * C], FP, kind="ExternalOutput")
    dbg_d = None
    if DEBUG:
        dbg_d = nc.dram_tensor("dbg", [P, 2 * C * F], FP, kind="ExternalOutput")

    V = nc.vector
    G = nc.gpsimd
    S = nc.scalar

    uid = [0]

    def mk(pool, cls):
        def t(_tag=None):
            uid[0] += 1
            fe, bufs, dt = _CLS[cls]
            return pool.tile([P, fe], dt, tag=cls, bufs=bufs,
                             name=f"{cls}_{uid[0]}")[:]
        return t

    with tile.TileContext(nc) as tc:
        with tc.tile_pool(name="pers", bufs=1) as pers, \
             tc.tile_pool(name="work", bufs=1) as wp:
            tF = mk(wp, "tF")
            t4F = mk(wp, "t4F")
            bF = mk(wp, "bF")
            b4F = mk(wp, "b4F")
            b8F = mk(wp, "b8F")

            def PT(tag, shape=None, dt=FP):
                return pers.tile(shape or [P, F], dt, tag=tag, name=tag)[:]

            def view(ap, g):
                return ap.rearrange("p (g f) -> p g f", g=g)

            def bc(plane, g):
                return plane.rearrange("p (o f) -> p o f", o=1).broadcast_to([P, g, F])

            halfpi = PT("halfpi", [P, 1])
            V.memset(halfpi, PI / 2)

            consts = {}

            def cplane(c):
                if c not in consts:
                    t = pers.tile([P, 1], FP, tag=f"c_{len(consts)}",
                                  name=f"c_{len(consts)}")[:]
                    V.memset(t, float(c))
                    consts[c] = t
                return consts[c]

            def cb(c, like):
                t = cplane(c)
                shp = list(like.shape)
                pat = "p (" + " ".join(f"d{i}" for i in range(len(shp) - 1)) + ") -> p " + \
                      " ".join(f"d{i}" for i in range(len(shp) - 1))
                kw = {f"d{i}": 1 for i in range(len(shp) - 1)}
                return t.rearrange(pat, **kw).broadcast_to(shp)

            sumcnt = pers.tile([P, 2 * C], FP, tag="sumcnt", name="sumcnt")[:]

            for k in range(C):
                _emit_chunk(nc, tc, wp, pers, k, V, G, S, tF, t4F, bF, b4F, b8F,
                            PT, view, bc, cb, halfpi, sumcnt,
                            pred_d, tgt_d, iou_d, dbg_d)

            nc.sync.dma_start(out_d.ap(), sumcnt)

    nc.compile()
    return nc


def _emit_chunk(nc, tc, wp, pers, k, V, G, S, tF, t4F, bF, b4F, b8F,
                PT, view, bc, cb, halfpi, sumcnt,
                pred_d, tgt_d, iou_d, dbg_d=None):
    FPc = FP

    def relu(eng_s, out, in_):
        eng_s.activation(out, in_, ACTF.Relu)

    # ======== stage 1: DMA + per-box primitives ========
    predI = wp.tile([P, 7 * F], FPc, tag="predI", bufs=1, name=f"predI_{k}")[:]
    tgtI = wp.tile([P, 7 * F], FPc, tag="tgtI", bufs=1, name=f"tgtI_{k}")[:]
    iouP = wp.tile([P, F], FPc, tag="iouP", bufs=1, name=f"iouP_{k}")[:]
    sl = slice(k * BLK, (k + 1) * BLK)
    nc.sync.dma_start(predI, pred_d.ap()[sl, :].rearrange("(p f) c -> p (f c)", p=P))
    nc.sync.dma_start(tgtI, tgt_d.ap()[sl, :].rearrange("(p f) c -> p (f c)", p=P))
    nc.sync.dma_start(iouP, iou_d.ap()[sl].rearrange("(p f) -> p f", p=P))

    pv = predI.rearrange("p (f c) -> p c f", c=7)
    tv = tgtI.rearrange("p (f c) -> p c f", c=7)
    x1, y1, z1, w1, l1, h1, yaw1 = (pv[:, c, :] for c in range(7))
    x2, y2, z2, w2, l2, h2, yaw2 = (tv[:, c, :] for c in range(7))

    def sincos(eng, yaw, pfx):
        is_v = eng is V
        g1 = tF()
        V.tensor_scalar(g1, yaw, PI, None, op0=OP.is_gt)
        g2 = tF()
        V.tensor_scalar(g2, yaw, -PI, None, op0=OP.is_lt)
        adj = tF()
        eng.tensor_tensor(adj, g2, g1, op=OP.subtract)
        yr = tF()
        if is_v:
            eng.scalar_tensor_tensor(yr, adj, 2 * PI, yaw, op0=OP.mult, op1=OP.add)
        else:
            tmp = tF()
            eng.tensor_tensor(tmp, adj, cb(2 * PI, tmp), op=OP.mult)
            eng.tensor_tensor(yr, tmp, yaw, op=OP.add)
        sa = PT(pfx + "sa")
        S.activation(sa, yr, ACTF.Sin)
        g3 = tF()
        V.tensor_scalar(g3, yr, PI / 2, None, op0=OP.is_gt)
        yc = tF()
        if is_v:
            eng.scalar_tensor_tensor(yc, g3, -2 * PI, yr, op0=OP.mult, op1=OP.add)
        else:
            tmp2 = tF()
            eng.tensor_tensor(tmp2, g3, cb(-2 * PI, tmp2), op=OP.mult)
            eng.tensor_tensor(yc, tmp2, yr, op=OP.add)
        ca = PT(pfx + "ca")
        S.activation(ca, yc, ACTF.Sin, bias=halfpi)
        return sa, ca

    sa1, ca1 = sincos(V, yaw1, "t1")
    sa2, ca2 = sincos(G, yaw2, "t2")

    cx2 = PT("cx2")
    G.tensor_tensor(cx2, x2, x1, op=OP.subtract)
    cy2 = PT("cy2")
    G.tensor_tensor(cy2, y2, y1, op=OP.subtract)

    def halfaxes(eng, w, l, sa, ca, r):
        if eng is V:
            A = PT(f"A{r}")
            eng.scalar_tensor_tensor(A, w, 0.5, ca, op0=OP.mult, op1=OP.mult)
            B = PT(f"B{r}")
            eng.scalar_tensor_tensor(B, w, 0.5, sa, op0=OP.mult, op1=OP.mult)
            Cc = tF()
            eng.scalar_tensor_tensor(Cc, l, 0.5, sa, op0=OP.mult, op1=OP.mult)
            D = PT(f"D{r}")
            eng.scalar_tensor_tensor(D, l, 0.5, ca, op0=OP.mult, op1=OP.mult)
        else:
            hw = tF()
            S.mul(hw, w, 0.5)
            hl = tF()
            S.mul(hl, l, 0.5)
            A = PT(f"A{r}")
            eng.tensor_tensor(A, hw, ca, op=OP.mult)
            B = PT(f"B{r}")
            eng.tensor_tensor(B, hw, sa, op=OP.mult)
            Cc = tF()
            eng.tensor_tensor(Cc, hl, sa, op=OP.mult)
            D = PT(f"D{r}")
            eng.tensor_tensor(D, hl, ca, op=OP.mult)
        nC = PT(f"nC{r}")
        S.mul(nC, Cc, -1.0)
        Pp = PT(f"P{r}")
        eng.tensor_tensor(Pp, A, Cc, op=OP.subtract)
        Q = PT(f"Q{r}")
        eng.tensor_tensor(Q, B, D, op=OP.add)
        R = PT(f"R{r}")
        eng.tensor_tensor(R, A, Cc, op=OP.add)
        Ss = PT(f"S{r}")
        eng.tensor_tensor(Ss, B, D, op=OP.subtract)
        return A, B, D, nC, Pp, Q, R, Ss

    A1, B1, D1, nC1, P1, Q1, R1, S1 = halfaxes(V, w1, l1, sa1, ca1, 1)
    A2, B2, D2, nC2, P2, Q2, R2, S2 = halfaxes(G, w2, l2, sa2, ca2, 2)

    # z overlap / volumes / mask
    hh1 = tF()
    S.mul(hh1, h1, 0.5)
    hh2 = tF()
    S.mul(hh2, h2, 0.5)
    zmax1 = tF()
    G.tensor_tensor(zmax1, z1, hh1, op=OP.add)
    zmin1 = tF()
    G.tensor_tensor(zmin1, z1, hh1, op=OP.subtract)
    zmax2 = tF()
    G.tensor_tensor(zmax2, z2, hh2, op=OP.add)
    zmin2 = tF()
    G.tensor_tensor(zmin2, z2, hh2, op=OP.subtract)
    mn_hi = tF()
    V.tensor_tensor(mn_hi, zmax1, zmax2, op=OP.min)
    mx_lo = tF()
    V.tensor_tensor(mx_lo, zmin1, zmin2, op=OP.max)
    ozr = tF()
    G.tensor_tensor(ozr, mn_hi, mx_lo, op=OP.subtract)
    oz = PT("oz")
    relu(S, oz, ozr)
    mx_hi = tF()
    V.tensor_tensor(mx_hi, zmax1, zmax2, op=OP.max)
    mn_lo = tF()
    V.tensor_tensor(mn_lo, zmin1, zmin2, op=OP.min)
    zrr = tF()
    G.tensor_tensor(zrr, mx_hi, mn_lo, op=OP.subtract)
    zr = PT("zr")
    relu(S, zr, zrr)

    v1a = tF()
    G.tensor_tensor(v1a, w1, l1, op=OP.mult)
    v1v = PT("v1v")
    G.tensor_tensor(v1v, v1a, h1, op=OP.mult)
    v2a = tF()
    G.tensor_tensor(v2a, w2, l2, op=OP.mult)
    v2v = PT("v2v")
    G.tensor_tensor(v2v, v2a, h2, op=OP.mult)
    mask = PT("mask")
    V.tensor_scalar(mask, iouP, 0.55, None, op0=OP.is_ge)

    hw1sq = PT("hw1sq")
    V.scalar_tensor_tensor(hw1sq, w1, 0.25, w1, op0=OP.mult, op1=OP.mult)
    hl1sq = PT("hl1sq")
    V.scalar_tensor_tensor(hl1sq, l1, 0.25, l1, op0=OP.mult, op1=OP.mult)
    hw2sq = PT("hw2sq")
    V.scalar_tensor_tensor(hw2sq, w2, 0.25, w2, op0=OP.mult, op1=OP.mult)
    hl2sq = PT("hl2sq")
    V.scalar_tensor_tensor(hl2sq, l2, 0.25, l2, op0=OP.mult, op1=OP.mult)
    hwl1 = PT("hwl1")
    V.scalar_tensor_tensor(hwl1, w1, 0.25, l1, op0=OP.mult, op1=OP.mult)
    hwl2 = PT("hwl2")
    V.scalar_tensor_tensor(hwl2, w2, 0.25, l2, op0=OP.mult, op1=OP.mult)

    il1 = tF()
    V.reciprocal(il1, l1)
    rat1 = PT("rat1")
    V.tensor_tensor(rat1, w1, il1, op=OP.mult)
    iw1 = tF()
    V.reciprocal(iw1, w1)
    irat1 = PT("irat1")
    V.tensor_tensor(irat1, l1, iw1, op=OP.mult)
    il2 = tF()
    V.reciprocal(il2, l2)
    rat2 = PT("rat2")
    V.tensor_tensor(rat2, w2, il2, op=OP.mult)
    iw2 = tF()
    V.reciprocal(iw2, w2)
    irat2 = PT("irat2")
    V.tensor_tensor(irat2, l2, iw2, op=OP.mult)

    # ---- bf16 working copies of the shared primitives (Act converts) ----
    def BPT(tag, srcv):
        o = PT("b" + tag, dt=BF)
        S.copy(o, srcv)
        return o

    bA1 = BPT("A1", A1); bB1 = BPT("B1", B1); bD1 = BPT("D1", D1); bnC1 = BPT("nC1", nC1)
    bP1 = BPT("P1", P1); bQ1 = BPT("Q1", Q1); bR1 = BPT("R1", R1); bS1 = BPT("S1", S1)
    bA2 = BPT("A2", A2); bB2 = BPT("B2", B2); bD2 = BPT("D2", D2); bnC2 = BPT("nC2", nC2)
    bP2 = BPT("P2", P2); bQ2 = BPT("Q2", Q2); bR2 = BPT("R2", R2); bS2 = BPT("S2", S2)
    bcx2 = BPT("cx2", cx2); bcy2 = BPT("cy2", cy2)
    brat1 = BPT("rat1", rat1); birat1 = BPT("irat1", irat1)
    brat2 = BPT("rat2", rat2); birat2 = BPT("irat2", irat2)

    # ---- dots needed downstream (outputs persistent) ----
    def dot(eng, tag, ax, ay, bx, by):
        t0 = tF()
        eng.tensor_tensor(t0, ax, bx, op=OP.mult)
        t1 = tF()
        eng.tensor_tensor(t1, ay, by, op=OP.mult)
        o = PT(tag)
        eng.tensor_tensor(o, t0, t1, op=OP.add)
        return o

    def dott(eng, ax, ay, bx, by):
        t0 = tF()
        eng.tensor_tensor(t0, ax, bx, op=OP.mult)
        t1 = tF()
        eng.tensor_tensor(t1, ay, by, op=OP.mult)
        o = tF()
        eng.tensor_tensor(o, t0, t1, op=OP.add)
        return o

    def saferec(tag, m):
        g = tF()
        V.tensor_scalar(g, m, 0.0, None, op0=OP.is_ge)
        s2 = tF()
        V.tensor_scalar(s2, g, 2.0, 1.0, op0=OP.mult, op1=OP.subtract)
        am = PT(tag + "_am")
        S.activation(am, m, ACTF.Abs)
        amc = tF()
        V.tensor_scalar(amc, am, 1e-12, None, op0=OP.max)
        ms = tF()
        V.tensor_tensor(ms, s2, amc, op=OP.mult)
        o = PT(tag)
        V.reciprocal(o, ms)
        return o, am

    inv_uu, am_uu = saferec("inv_uu", dott(V, A2, B2, A1, B1))
    inv_uv, am_uv = saferec("inv_uv", dott(V, A2, B2, nC1, D1))
    inv_vu, am_vu = saferec("inv_vu", dott(V, nC2, D2, A1, B1))
    inv_vv, am_vv = saferec("inv_vv", dott(V, nC2, D2, nC1, D1))

    def bdot(eng, tag, ax, ay, bx, by):
        t0 = bF()
        eng.tensor_tensor(t0, ax, bx, op=OP.mult)
        t1 = bF()
        eng.tensor_tensor(t1, ay, by, op=OP.mult)
        o = PT(tag, dt=BF)
        eng.tensor_tensor(o, t0, t1, op=OP.add)
        return o

    pj = {}
    for axname, axx, axy, eng in (
        ("u1", bA1, bB1, V),
        ("v1", bnC1, bD1, V),
        ("u2", bA2, bB2, G),
        ("v2", bnC2, bD2, G),
    ):
        for vec, vx, vy in (
            ("PQ1", bP1, bQ1),
            ("RS1", bR1, bS1),
            ("PQ2", bP2, bQ2),
            ("RS2", bR2, bS2),
            ("C", bcx2, bcy2),
        ):
            pj[(axname, vec)] = bdot(eng, f"pj_{axname}_{vec}", axx, axy, vx, vy)

    # X_u = cx2*B2 - cy2*A2 ; X_v = cx2*D2 + cy2*C2
    xu0 = tF()
    G.tensor_tensor(xu0, cx2, B2, op=OP.mult)
    xu1 = tF()
    G.tensor_tensor(xu1, cy2, A2, op=OP.mult)
    X_u = PT("X_u")
    G.tensor_tensor(X_u, xu0, xu1, op=OP.subtract)
    xv0 = tF()
    G.tensor_tensor(xv0, cx2, D2, op=OP.mult)
    xv1 = tF()
    G.tensor_tensor(xv1, cy2, nC2, op=OP.mult)
    X_v = PT("X_vf")
    G.tensor_tensor(X_v, xv0, xv1, op=OP.subtract)  # cx2*D2 + cy2*C2

    # ======== stage 2: intersection ========
    def corner_su(eng, dPQ, dRS, dC, sign_off):
        outs = []
        for (src, sgn) in ((dPQ, 1), (dRS, -1), (dPQ, -1), (dRS, 1)):
            o = bF()
            if sign_off < 0:
                if sgn > 0:
                    eng.tensor_tensor(o, src, dC, op=OP.subtract)
                else:
                    if eng is V:
                        eng.scalar_tensor_tensor(o, src, -1.0, dC, op0=OP.mult, op1=OP.subtract)
                    else:
                        t = bF()
                        S.mul(t, src, -1.0)
                        eng.tensor_tensor(o, t, dC, op=OP.subtract)
            else:
                if sgn > 0:
                    eng.tensor_tensor(o, src, dC, op=OP.add)
                else:
                    eng.tensor_tensor(o, dC, src, op=OP.subtract)
            outs.append(o)
        return outs

    su1u = corner_su(V, pj[("u2", "PQ1")], pj[("u2", "RS1")], pj[("u2", "C")], -1)
    su1v = corner_su(V, pj[("v2", "PQ1")], pj[("v2", "RS1")], pj[("v2", "C")], -1)
    su2u = corner_su(G, pj[("u1", "PQ2")], pj[("u1", "RS2")], pj[("u1", "C")], +1)
    su2v = corner_su(G, pj[("v1", "PQ2")], pj[("v1", "RS2")], pj[("v1", "C")], +1)

    def emit_pass(eng, su_by_axis, h_by_axis, inv_by_edge_axis):
        is_v = eng is V
        suA, suB = su_by_axis
        hA, hB = h_by_axis
        su_s = b8F()
        suv = su_s.rearrange("p (e a f) -> p e a f", e=4, a=2)
        inv_s = b8F()
        invv = inv_s.rearrange("p (e a f) -> p e a f", e=4, a=2)
        h_s = b4F()
        hv = view(h_s, 4)
        S.copy(hv[:, 0, :], hA)
        S.copy(hv[:, 1, :], hB)
        for e in range(4):
            S.copy(suv[:, e, 0, :], suA[e])
            S.copy(suv[:, e, 1, :], suB[e])
            for a in range(2):
                ip, cf = inv_by_edge_axis[e][a]
                # for POOL: store NEGATED inv so r1 = (su+h)*(-inv)
                S.mul(invv[:, e, a, :], ip, cf if is_v else -cf)
        hb = (h_s[:, 0:2 * F]
              .rearrange("p (o a f) -> p o a f", o=1, a=2)
              .broadcast_to([P, 4, 2, F]))
        sus = su_s.rearrange("p (e a f) -> p e a f", e=4, a=2)
        a1 = b8F()
        if is_v:
            eng.scalar_tensor_tensor(a1.rearrange("p (e a f) -> p e a f", e=4, a=2),
                                     sus, -1.0, hb, op0=OP.mult, op1=OP.subtract)
        else:
            # a1 = su + h ; combined with negated inv gives same r1
            eng.tensor_tensor(a1.rearrange("p (e a f) -> p e a f", e=4, a=2),
                              sus, hb, op=OP.add)
        a2 = b8F()
        if is_v:
            eng.tensor_tensor(a2.rearrange("p (e a f) -> p e a f", e=4, a=2),
                              hb, sus, op=OP.subtract)
        else:
            # r2 = (h-su)*inv = (su-h)*(-inv); inv strip holds -inv
            eng.tensor_tensor(a2.rearrange("p (e a f) -> p e a f", e=4, a=2),
                              sus, hb, op=OP.subtract)
        r1 = b8F()
        eng.tensor_tensor(r1, a1, inv_s, op=OP.mult)
        r2 = b8F()
        eng.tensor_tensor(r2, a2, inv_s, op=OP.mult)
        lo = b8F()
        V.tensor_tensor(lo, r1, r2, op=OP.min)
        hi = b8F()
        V.tensor_tensor(hi, r1, r2, op=OP.max)
        lov = lo.rearrange("p (e a f) -> p e a f", e=4, a=2)
        hiv = hi.rearrange("p (e a f) -> p e a f", e=4, a=2)
        t0p = b4F()
        V.tensor_tensor(view(t0p, 4), lov[:, :, 0, :], lov[:, :, 1, :], op=OP.max)
        t0 = b4F()
        relu(S, t0, t0p)
        t1p = b4F()
        V.tensor_tensor(view(t1p, 4), hiv[:, :, 0, :], hiv[:, :, 1, :], op=OP.min)
        t1 = b4F()
        V.tensor_scalar(t1, t1p, 1.0, None, op0=OP.min)
        dt = b4F()
        eng.tensor_tensor(dt, t1, t0, op=OP.subtract)
        dtc = b4F()
        relu(S, dtc, dt)
        return dtc

    inv1 = [
        [(inv_uu, -0.5), (inv_vu, -0.5)],
        [(inv_uv, -0.5), (inv_vv, -0.5)],
        [(inv_uu, 0.5), (inv_vu, 0.5)],
        [(inv_uv, 0.5), (inv_vv, 0.5)],
    ]
    dt1 = emit_pass(V, (su1u, su1v), (hw2sq, hl2sq), inv1)
    inv2 = [
        [(inv_uu, -0.5), (inv_uv, -0.5)],
        [(inv_vu, -0.5), (inv_vv, -0.5)],
        [(inv_uu, 0.5), (inv_uv, 0.5)],
        [(inv_vu, 0.5), (inv_vv, 0.5)],
    ]
    dt2 = emit_pass(G, (su2u, su2v), (hw1sq, hl1sq), inv2)

    dt1v = view(dt1, 4)
    sa_ = b4F()
    V.tensor_tensor(view(sa_, 4)[:, 0:2, :], dt1v[:, 0:2, :], dt1v[:, 2:4, :], op=OP.add)
    sav = view(sa_, 4)
    sum1 = bF()
    V.tensor_tensor(sum1, sav[:, 0, :], sav[:, 1, :], op=OP.add)
    contrib1 = tF()
    V.tensor_tensor(contrib1, sum1, hwl1, op=OP.mult)

    dt2v = view(dt2, 4)
    sb_ = b4F()
    G.tensor_tensor(view(sb_, 4)[:, 0:2, :], dt2v[:, 0:2, :], dt2v[:, 2:4, :], op=OP.add)
    sbv = view(sb_, 4)
    sum2 = bF()
    G.tensor_tensor(sum2, sbv[:, 0, :], sbv[:, 1, :], op=OP.add)
    base2 = tF()
    G.tensor_tensor(base2, sum2, hwl2, op=OP.mult)
    d20 = bF()
    G.tensor_tensor(d20, dt2v[:, 2, :], dt2v[:, 0, :], op=OP.subtract)
    d31 = bF()
    G.tensor_tensor(d31, dt2v[:, 3, :], dt2v[:, 1, :], op=OP.subtract)
    tXu = tF()
    G.tensor_tensor(tXu, d20, X_u, op=OP.mult)
    tXv = tF()
    G.tensor_tensor(tXv, d31, X_v, op=OP.mult)
    c2s = tF()
    G.tensor_tensor(c2s, base2, tXu, op=OP.add)
    c2t = tF()
    G.tensor_tensor(c2t, c2s, tXv, op=OP.add)
    isum = tF()
    V.tensor_tensor(isum, contrib1, c2t, op=OP.add)
    inter2d = PT("inter2d")
    V.scalar_tensor_tensor(inter2d, isum, -1.0, isum, op0=OP.mult, op1=OP.max)

    # ======== stage 3: enclosing rectangle ========
    ox = wp.tile([P, 4 * F], BF, tag="ox", name=f"ox_{k}")[:]
    oxv = view(ox, 4)
    S.copy(oxv[:, 0, :], P1)
    S.mul(oxv[:, 1, :], R1, -1.0)
    S.mul(oxv[:, 2, :], P1, -1.0)
    S.copy(oxv[:, 3, :], R1)
    oy = wp.tile([P, 4 * F], BF, tag="oy", name=f"oy_{k}")[:]
    oyv = view(oy, 4)
    S.copy(oyv[:, 0, :], Q1)
    S.mul(oyv[:, 1, :], S1, -1.0)
    S.mul(oyv[:, 2, :], Q1, -1.0)
    S.copy(oyv[:, 3, :], S1)
    pos = {}
    for ax in ("u1", "v1", "u2", "v2"):
        st = wp.tile([P, 4 * F], BF, tag=f"po_{ax}", name=f"po_{ax}_{k}")[:]
        sv = view(st, 4)
        dPQ1 = pj[(ax, "PQ1")]
        dRS1 = pj[(ax, "RS1")]
        S.copy(sv[:, 0, :], dPQ1)
        S.mul(sv[:, 1, :], dRS1, -1.0)
        S.mul(sv[:, 2, :], dPQ1, -1.0)
        S.copy(sv[:, 3, :], dRS1)
        pos[ax] = sv

    encmin = wp.tile([P, 4 * F], FPc, tag="encmin", name=f"encmin_{k}")[:]
    encminv = view(encmin, 4)

    # per-corner-j group of 4 cross directions; alternate V / G per group
    for j in range(4):
        E = V if j < 3 else G
        sP, sR = ((1, 0), (-1, 1), (-1, 0), (1, 1))[j]
        # rect2 corner j = ctr2 + sgn*(P2,Q2) or sgn*(R2,S2)
        wxp = bF()
        wyp = bF()
        if sR == 0:
            if sP > 0:
                E.tensor_tensor(wxp, bcx2, bP2, op=OP.add)
                E.tensor_tensor(wyp, bcy2, bQ2, op=OP.add)
            else:
                E.tensor_tensor(wxp, bcx2, bP2, op=OP.subtract)
                E.tensor_tensor(wyp, bcy2, bQ2, op=OP.subtract)
        else:
            if sP > 0:
                E.tensor_tensor(wxp, bcx2, bR2, op=OP.add)
                E.tensor_tensor(wyp, bcy2, bS2, op=OP.add)
            else:
                E.tensor_tensor(wxp, bcx2, bR2, op=OP.subtract)
                E.tensor_tensor(wyp, bcy2, bS2, op=OP.subtract)
        # pw values for the 4 axes at this corner
        pwj = {}
        for ax in ("u1", "v1", "u2", "v2"):
            o = bF()
            dC = pj[(ax, "C")]
            src = pj[(ax, "PQ2")] if sR == 0 else pj[(ax, "RS2")]
            if sP > 0:
                E.tensor_tensor(o, dC, src, op=OP.add)
            else:
                E.tensor_tensor(o, dC, src, op=OP.subtract)
            pwj[ax] = o

        def lin(ax):
            o = view(b4F(), 4)
            E.tensor_tensor(o, bc(pwj[ax], 4), pos[ax], op=OP.subtract)
            return o

        du1 = lin("u1")
        dv1 = lin("v1")
        du2 = lin("u2")
        dv2 = lin("v2")

        def aabs(x):
            o = view(b4F(), 4)
            S.activation(o, x, ACTF.Abs)
            return o

        adu1 = aabs(du1)
        adv1 = aabs(dv1)
        adu2 = aabs(du2)
        adv2 = aabs(dv2)
        h1d = view(b4F(), 4)
        E.tensor_tensor(h1d, adu1, adv1, op=OP.add)
        h2d = view(b4F(), 4)
        E.tensor_tensor(h2d, adu2, adv2, op=OP.add)
        h1p0 = view(b4F(), 4)
        E.tensor_tensor(h1p0, bc(brat1, 4), adv1, op=OP.mult)
        h1p1 = view(b4F(), 4)
        E.tensor_tensor(h1p1, bc(birat1, 4), adu1, op=OP.mult)
        h1p = view(b4F(), 4)
        E.tensor_tensor(h1p, h1p0, h1p1, op=OP.add)
        h2p0 = view(b4F(), 4)
        E.tensor_tensor(h2p0, bc(brat2, 4), adv2, op=OP.mult)
        h2p1 = view(b4F(), 4)
        E.tensor_tensor(h2p1, bc(birat2, 4), adu2, op=OP.mult)
        h2p = view(b4F(), 4)
        E.tensor_tensor(h2p, h2p0, h2p1, op=OP.add)

        ED = V if j == 3 else E
        dx = view(b4F(), 4)
        ED.tensor_tensor(dx, bc(wxp, 4), oxv, op=OP.subtract)
        dy = view(b4F(), 4)
        ED.tensor_tensor(dy, bc(wyp, 4), oyv, op=OP.subtract)
        dc0 = view(b4F(), 4)
        ED.tensor_tensor(dc0, dx, bc(bcx2, 4), op=OP.mult)
        dc1 = view(b4F(), 4)
        ED.tensor_tensor(dc1, dy, bc(bcy2, 4), op=OP.mult)
        dcv = view(b4F(), 4)
        ED.tensor_tensor(dcv, dc0, dc1, op=OP.add)
        dp0 = view(b4F(), 4)
        ED.tensor_tensor(dp0, dx, bc(bcy2, 4), op=OP.mult)
        dp1 = view(b4F(), 4)
        ED.tensor_tensor(dp1, dy, bc(bcx2, 4), op=OP.mult)
        dcp = view(b4F(), 4)
        ED.tensor_tensor(dcp, dp0, dp1, op=OP.subtract)
        sqx = view(t4F(), 4)
        S.activation(sqx, dx, ACTF.Square)
        sqy = view(t4F(), 4)
        S.activation(sqy, dy, ACTF.Square)
        dd = view(t4F(), 4)
        E.tensor_tensor(dd, sqx, sqy, op=OP.add)

        def rng(hA, hB, dcx):
            ee1 = view(b4F(), 4)
            E.tensor_tensor(ee1, dcx, hB, op=OP.add)
            mm1 = view(b4F(), 4)
            V.tensor_tensor(mm1, hA, ee1, op=OP.max)
            ee2 = view(b4F(), 4)
            E.tensor_tensor(ee2, hB, dcx, op=OP.subtract)
            mm2 = view(b4F(), 4)
            V.tensor_tensor(mm2, hA, ee2, op=OP.max)
            o = view(b4F(), 4)
            E.tensor_tensor(o, mm1, mm2, op=OP.add)
            return o

        rng_d = rng(h1d, h2d, dcv)
        rng_p = rng(h1p, h2p, dcp)
        ar = view(t4F(), 4)
        E.tensor_tensor(ar, rng_d, rng_p, op=OP.mult)
        dds = view(t4F(), 4)
        V.tensor_scalar(dds, dd, 1e-30, None, op0=OP.max)
        inv = view(t4F(), 4)
        V.reciprocal(inv, dds)
        ar2 = view(t4F(), 4)
        E.tensor_tensor(ar2, ar, inv, op=OP.mult)
        le = view(t4F(), 4)
        V.tensor_scalar(le, dd, 0.25, None, op0=OP.is_le)
        if j == 0:
            V.scalar_tensor_tensor(encminv, le, 1e18, ar2, op0=OP.mult, op1=OP.add)
        else:
            ar3 = view(t4F(), 4)
            V.scalar_tensor_tensor(ar3, le, 1e18, ar2, op0=OP.mult, op1=OP.add)
            V.tensor_tensor(encminv, encminv, ar3, op=OP.min)

    # --- rect-edge directions (4) ---
    red_dd = view(t4F(), 4)
    S.copy(red_dd[:, 0, :], hw1sq)
    S.copy(red_dd[:, 1, :], hl1sq)
    S.copy(red_dd[:, 2, :], hw2sq)
    S.copy(red_dd[:, 3, :], hl2sq)
    red_hop = view(t4F(), 4)
    S.copy(red_hop[:, 0, :], hwl1)
    S.copy(red_hop[:, 1, :], hwl1)
    S.copy(red_hop[:, 2, :], hwl2)
    S.copy(red_hop[:, 3, :], hwl2)
    red_hod = view(t4F(), 4)
    G.tensor_tensor(red_hod[:, 0, :], am_uu, am_vu, op=OP.add)
    G.tensor_tensor(red_hod[:, 1, :], am_uv, am_vv, op=OP.add)
    G.tensor_tensor(red_hod[:, 2, :], am_uu, am_uv, op=OP.add)
    G.tensor_tensor(red_hod[:, 3, :], am_vu, am_vv, op=OP.add)
    red_hpp = view(t4F(), 4)
    for kk, (ra, ib, aa, ab) in enumerate((
        (rat2, irat2, am_vu, am_uu),
        (rat2, irat2, am_vv, am_uv),
        (rat1, irat1, am_uv, am_uu),
        (rat1, irat1, am_vv, am_vu),
    )):
        ta = tF()
        G.tensor_tensor(ta, ra, aa, op=OP.mult)
        tb = tF()
        G.tensor_tensor(tb, ib, ab, op=OP.mult)
        G.tensor_tensor(red_hpp[:, kk, :], ta, tb, op=OP.add)
    red_dc = view(t4F(), 4)
    for kk, ax in enumerate(("u1", "v1", "u2", "v2")):
        S.copy(red_dc[:, kk, :], pj[(ax, "C")])
    red_dcp = view(t4F(), 4)
    t0_ = tF()
    G.tensor_tensor(t0_, A1, cy2, op=OP.mult)
    t1_ = tF()
    G.tensor_tensor(t1_, B1, cx2, op=OP.mult)
    G.tensor_tensor(red_dcp[:, 0, :], t0_, t1_, op=OP.subtract)
    t2_ = tF()
    G.tensor_tensor(t2_, nC1, cy2, op=OP.mult)
    t3_ = tF()
    G.tensor_tensor(t3_, D1, cx2, op=OP.mult)
    G.tensor_tensor(red_dcp[:, 1, :], t2_, t3_, op=OP.subtract)
    S.copy(red_dcp[:, 2, :], X_u)
    S.copy(red_dcp[:, 3, :], X_v)

    def rng4(hA, hB, dcx):
        ee1 = view(t4F(), 4)
        G.tensor_tensor(ee1, dcx, hB, op=OP.add)
        mm1 = view(t4F(), 4)
        V.tensor_tensor(mm1, hA, ee1, op=OP.max)
        ee2 = view(t4F(), 4)
        G.tensor_tensor(ee2, hB, dcx, op=OP.subtract)
        mm2 = view(t4F(), 4)
        V.tensor_tensor(mm2, hA, ee2, op=OP.max)
        o = view(t4F(), 4)
        G.tensor_tensor(o, mm1, mm2, op=OP.add)
        return o

    r4d = rng4(red_dd, red_hod, red_dc)
    r4p = rng4(red_hop, red_hpp, red_dcp)
    ar4 = view(t4F(), 4)
    G.tensor_tensor(ar4, r4d, r4p, op=OP.mult)
    inv4 = view(t4F(), 4)
    V.reciprocal(inv4, red_dd)
    ar4b = view(t4F(), 4)
    G.tensor_tensor(ar4b, ar4, inv4, op=OP.mult)
    V.tensor_tensor(encminv, encminv, ar4b, op=OP.min)

    m2_ = view(t4F(), 4)[:, 0:2, :]
    V.tensor_tensor(m2_, encminv[:, 0:2, :], encminv[:, 2:4, :], op=OP.min)
    vc_min = tF()
    V.tensor_tensor(vc_min, m2_[:, 0, :], m2_[:, 1, :], op=OP.min)

    if dbg_d is not None:
        nc.sync.dma_start(dbg_d.ap()[:, (2 * k) * F:(2 * k + 1) * F], inter2d)
        nc.sync.dma_start(dbg_d.ap()[:, (2 * k + 1) * F:(2 * k + 2) * F], vc_min)

    # ======== stage 4: loss + reduce ========
    inter3d = tF()
    V.tensor_tensor(inter3d, inter2d, oz, op=OP.mult)
    usum = tF()
    G.tensor_tensor(usum, v1v, v2v, op=OP.add)
    union = tF()
    V.tensor_tensor(union, usum, inter3d, op=OP.subtract)
    um = tF()
    V.tensor_scalar(um, union, 1e-8, None, op0=OP.max)
    ru = tF()
    V.reciprocal(ru, um)
    iou3 = tF()
    V.tensor_tensor(iou3, inter3d, ru, op=OP.mult)
    vc = tF()
    G.tensor_tensor(vc, vc_min, zr, op=OP.mult)
    vcm = tF()
    V.tensor_scalar(vcm, vc, 1e-8, None, op0=OP.max)
    rvc = tF()
    V.reciprocal(rvc, vcm)
    tv_ = tF()
    G.tensor_tensor(tv_, union, rvc, op=OP.mult)
    sm = tF()
    V.tensor_tensor(sm, iou3, tv_, op=OP.add)
    giou = tF()
    V.tensor_scalar(giou, sm, -1.0, 2.0, op0=OP.mult, op1=OP.add)
    lm = tF()
    V.scalar_tensor_tensor(lm, giou, 1.0, mask, op0=OP.mult, op1=OP.mult,
                           accum_out=sumcnt[:, 2 * k:2 * k + 1])
    cnt_dummy = tF()
    V.tensor_scalar(cnt_dummy, mask, 1.0, 0.0, op0=OP.mult, op1=OP.add,
                    accum_out=sumcnt[:, 2 * k + 1:2 * k + 2])


_NC = None


def _get_nc():
    global _NC
    if _NC is None:
        _NC = _build()
    return _NC


def kernel(pred: np.ndarray, target: np.ndarray, iou: np.ndarray) -> np.ndarray:
    nc = _get_nc()
    in_maps = []
    for c in range(N_STREAMS):
        sl = slice(c * N_S, (c + 1) * N_S)
        in_maps.append({
            "pred": np.ascontiguousarray(pred[sl], dtype=np.float32),
            "target": np.ascontiguousarray(target[sl], dtype=np.float32),
            "iou": np.ascontiguousarray(iou[sl], dtype=np.float32),
        })
    res = bass_utils.run_bass_kernel_spmd(nc, in_maps, core_ids=list(range(N_STREAMS)))
    tot = 0.0
    cnt = 0.0
    for r in res.results:
        pr = r["partials"].reshape(P, C, 2)
        tot += float(pr[:, :, 0].astype(np.float64).sum())
        cnt += float(pr[:, :, 1].astype(np.float64).sum())
    out = tot / max(cnt, 1.0) if cnt > 0 else 0.0
    return np.float32(out)
